# revision 1
# baseline (speedup 1.0000x reference)
"""Multi-head causal attention (B=2, T=2048, C=1024, H=16, S=64) on 8 TRN2 cores.

Sharding: core i handles batch b = i//4 and head group g = i%4 (4 heads each).
Each core computes a partial output projection (its heads' contribution to the
full [T, C] output); the host sums the 4 partials per batch and adds the bias.

Per-core dataflow (all layouts chosen so no on-chip transposes are needed;
bf16 matmuls with f32 PSUM accumulation throughout):
  qT/kT   [S, T]  = W.T @ x.T        (head-pair packed in the 128 partitions)
  v       [T, S]                     (bf16 stored, ones column appended for d)
  attT    [Tk, Tq] = kT-tile.T @ qT  (K=64; exact-causal tiles only)
  p       = exp(0.125 * attT)        (ACT, bf16 out; diagonal masked via 0/1 mul)
  yT|d    [S+1, Tq] = [v|1].T @ p    (row 64 = softmax denominator)
  yT_norm = yT * (1/d)               (reciprocal + partition_broadcast + mul)
  out     [T, C] partial = yT.T @ WpT (head-pair stacked contraction)
"""

import os
import math
import numpy as np
import ml_dtypes

import concourse.bacc as bacc
import concourse.mybir as mybir
import concourse.tile as tile
from concourse.bass_utils import run_bass_kernel_spmd

F32 = mybir.dt.float32
F32R = mybir.dt.float32r
BF16 = mybir.dt.bfloat16

B, T, C, H, S = 2, 2048, 1024, 16, 64
HPC = 4          # heads per core
N_CORES = 8
NC_T = T // 128  # 16 t-tiles of 128

# attT storage offsets: tile tk spans tq in [128*tk, 2048)
SPAN = [T - 128 * tk for tk in range(NC_T)]
OFF = [0] * NC_T
for _tk in range(1, NC_T):
    OFF[_tk] = OFF[_tk - 1] + SPAN[_tk - 1]
ATT_W = OFF[-1] + SPAN[-1]  # 17408

_cached_nc = None
last_results = None  # BassKernelResults of the most recent run (for test harness)


def _build():
    nc = bacc.Bacc("TRN2", target_bir_lowering=False)

    # bf16 QKV inputs, pre-chunked on host so each DMA is one big contiguous-
    # per-partition transfer (128 rows x 2-8KB): c-chunk c of wq[hp] lives at
    # cols [128c:128c+128], of wv at cols [256c:256c+256].
    xT_d = nc.dram_tensor("xT", [C, T], BF16, kind="ExternalInput")
    wq_d = nc.dram_tensor("wq", [2, 128, 8 * 128], BF16, kind="ExternalInput")
    wk_d = nc.dram_tensor("wk", [2, 128, 8 * 128], BF16, kind="ExternalInput")
    wv_d = nc.dram_tensor("wv", [128, 8 * 256], BF16, kind="ExternalInput")
    wpT_d = nc.dram_tensor("wpT", [2, 128, C], BF16, kind="ExternalInput")
    mask_d = nc.dram_tensor("mask", [128, 128], BF16, kind="ExternalInput")
    out_d = nc.dram_tensor("out", [T, C], BF16, kind="ExternalOutput")

    with tile.TileContext(nc) as tc:
        with (
            tc.tile_pool(name="const", bufs=1) as constp,
            tc.tile_pool(name="qkT", bufs=1) as qkp,
            tc.tile_pool(name="vsb", bufs=1) as vp,
            tc.tile_pool(name="yT", bufs=1) as ytp,
            tc.tile_pool(name="attT", bufs=1) as attp,
            tc.tile_pool(name="yps", bufs=2, space="PSUM") as yps,
            tc.tile_pool(name="sm", bufs=2) as smp,
        ):
            # persistent tiles
            mask_sb = constp.tile([128, 128], BF16, name="mask_sb")
            nc.sync.dma_start(mask_sb[:], mask_d[:])

            qT2 = [qkp.tile([128, T], BF16, name=f"qT2_{hp}") for hp in range(2)]
            kT2 = [qkp.tile([128, T], BF16, name=f"kT2_{hp}") for hp in range(2)]
            # v tiles: [128, 4*65] bf16; head h in cols 65h..65h+63, col 65h+64 = 1
            v_sb = [vp.tile([128, 4 * 65], BF16, name=f"v{t}") for t in range(NC_T)]
            for t in range(NC_T):
                ones_ap = v_sb[t].rearrange("p (h c) -> p h c", h=4)[:, :, 64]
                nc.vector.memset(ones_ap, 1.0)
            yT_all = [ytp.tile([128, T], BF16, name=f"yTa{hp}") for hp in range(2)]
            att_buf = [
                attp.tile([128, ATT_W], BF16, name=f"attb{i}") for i in range(3)
            ]
            BUF_OF = [0, 1, 2, 0]  # head -> attT buffer

            def emit_scores_tk(h, tk):
                hp, half = h // 2, h % 2
                r0 = 64 * half
                ab = att_buf[BUF_OF[h]]
                krow = kT2[hp][r0 : r0 + 64, :]
                qrow = qT2[hp][r0 : r0 + 64, :]
                span = SPAN[tk]
                kt = krow[:, 128 * tk : 128 * tk + 128]
                for part in range(math.ceil(span / 1024)):
                    pspan = min(1024, span - 1024 * part)
                    pt = sps.tile([128, 1024], F32, name="sps_t", tag="s")
                    for mmi in range(math.ceil(pspan / 512)):
                        n = min(512, pspan - 512 * mmi)
                        tq0 = 128 * tk + 1024 * part + 512 * mmi
                        nc.tensor.matmul(
                            pt[:, 512 * mmi : 512 * mmi + n],
                            kt,
                            qrow[:, tq0 : tq0 + n],
                            start=True,
                            stop=True,
                        )
                    dst = ab[
                        :, OFF[tk] + 1024 * part : OFF[tk] + 1024 * part + pspan
                    ]
                    nc.scalar.activation(
                        dst,
                        pt[:, 0:pspan],
                        mybir.ActivationFunctionType.Exp,
                        scale=0.125,
                    )
                # mask the diagonal block (first 128 cols of this tk tile)
                diag = ab[:, OFF[tk] : OFF[tk] + 128]
                nc.vector.tensor_mul(diag, diag, mask_sb[:])

            def emit_y_window(h, j):
                hp, half = h // 2, h % 2
                ab = att_buf[BUF_OF[h]]
                yp = yps.tile([65, 512], F32, name="yps_t", tag="y")
                tk_hi = min(NC_T - 1, 4 * j + 3)
                for tk in range(tk_hi + 1):
                    if 128 * tk <= 512 * j:
                        n = 512
                        outc = 0
                        ac = OFF[tk] + 512 * j - 128 * tk
                    else:
                        n = 512 * (j + 1) - 128 * tk
                        outc = 128 * tk - 512 * j
                        ac = OFF[tk]
                    nc.tensor.matmul(
                        yp[:, outc : outc + n],
                        v_sb[tk][:, 65 * h : 65 * h + 65],
                        ab[:, ac : ac + n],
                        start=(tk == 0),
                        stop=(tk == tk_hi),
                        skip_group_check=True,
                    )
                # normalize: yT_norm = yT * (1/d), d in psum row 64
                rec = smp.tile([1, 512], F32, name="rec")
                nc.vector.reciprocal(rec[:], yp[64:65, :])
                bc = smp.tile([64, 512], F32, name="bc")
                nc.gpsimd.partition_broadcast(bc[:], rec[:])
                dst = yT_all[hp][
                    64 * half : 64 * half + 64, 512 * j : 512 * j + 512
                ]
                if half == 0:
                    nc.vector.tensor_mul(dst, yp[0:64, :], bc[:])
                else:
                    stg = smp.tile([64, 512], BF16, name="stg")
                    nc.vector.tensor_mul(stg[:], yp[0:64, :], bc[:])
                    # SWDGE queue: keeps the partition shift off the HWDGE
                    # queue that carries the big input/output transfers.
                    nc.gpsimd.dma_start(dst, stg[:])

            # ---- scores/QKV scope: sps closes after phase E ----
            wpT_sb = [
                constp.tile([128, C], BF16, name=f"wpT{hp}") for hp in range(2)
            ]
            with (
                tc.tile_pool(name="sps", bufs=2, space="PSUM") as sps,
            ):
              with (
                tc.tile_pool(name="xw", bufs=1) as xw,
                tc.tile_pool(name="mmps", bufs=2, space="PSUM") as mmps,
              ):
                # x first (the QK c-loop consumes chunks in order), weights
                # adjacent to first use; all transfers are 128 x 2-8KB rows.
                wq_sb = [
                    xw.tile([128, 1024], BF16, name=f"wq{hp}") for hp in range(2)
                ]
                wk_sb = [
                    xw.tile([128, 1024], BF16, name=f"wk{hp}") for hp in range(2)
                ]
                wv_sb = xw.tile([128, 2048], BF16, name="wv")
                xT_sb = [xw.tile([128, T], BF16, name=f"xT{c}") for c in range(8)]
                nc.sync.dma_start(wq_sb[0][:], wq_d[0])
                # half-major loads: the first two QK groups only need
                # cols 0-1023 of every chunk, so they can start after ~2MB
                # of the 4MB x transfer instead of all of it.
                for half in range(2):
                    for c in range(8):
                        nc.sync.dma_start(
                            xT_sb[c][:, 1024 * half : 1024 * half + 1024],
                            xT_d[
                                128 * c : 128 * c + 128,
                                1024 * half : 1024 * half + 1024,
                            ],
                        )
                nc.sync.dma_start(wk_sb[0][:], wk_d[0])
                nc.sync.dma_start(wv_sb[:], wv_d[:])
                nc.sync.dma_start(wq_sb[1][:], wq_d[1])
                nc.sync.dma_start(wk_sb[1][:], wk_d[1])

                def emit_qk_group(hp, kind, tq):
                    w_sb = wq_sb if kind == 0 else wk_sb
                    dst = qT2[hp] if kind == 0 else kT2[hp]
                    pt = mmps.tile([128, 512], F32, name="qkps", tag="qk")
                    for c in range(8):
                        nc.tensor.matmul(
                            pt[:],
                            w_sb[hp][:, 128 * c : 128 * c + 128],
                            xT_sb[c][:, 512 * tq : 512 * tq + 512],
                            start=(c == 0),
                            stop=(c == 7),
                        )
                    nc.vector.tensor_copy(dst[:, 512 * tq : 512 * tq + 512], pt[:])

                def emit_v_t(t):
                    pv = mmps.tile([128, 256], F32, name="vps", tag="qk")
                    for c in range(8):
                        nc.tensor.matmul(
                            pv[:],
                            xT_sb[c][:, 128 * t : 128 * t + 128],
                            wv_sb[:, 256 * c : 256 * c + 256],
                            start=(c == 0),
                            stop=(c == 7),
                        )
                    nc.vector.tensor_copy(
                        v_sb[t].rearrange("p (h c) -> p h c", h=4)[:, :, 0:64],
                        pv[:].rearrange("p (h c) -> p h c", h=4),
                    )

                # PE warm-up: dummy matmuls on the mask tile while the
                # first input DMAs are in flight (HAM clock-gate warm-up).
                warm = sps.tile([128, 1024], F32, name="warm", tag="s")
                for i in range(24):
                    nc.tensor.matmul(
                        warm[:, 0:128],
                        mask_sb[:],
                        mask_sb[:],
                        start=True,
                        stop=True,
                    )
                # Phase A: q projections for head-pair 0.
                for tq in range(4):
                    emit_qk_group(0, 0, tq)
                for hp in range(2):
                    nc.gpsimd.dma_start(wpT_sb[hp][:], wpT_d[hp])
                # Phase B: k(hp0) + scores h0 + q(hp1) filler.
                for g in range(4):
                    emit_qk_group(0, 1, g)
                    for tk in range(4 * g, 4 * g + 4):
                        emit_scores_tk(0, tk)
                    emit_qk_group(1, 0, g)
                # Phase C: k(hp1) + scores h1 + first half of v.
                for g in range(4):
                    emit_qk_group(1, 1, g)
                    for tk in range(4 * g, 4 * g + 4):
                        emit_scores_tk(1, tk)
                    emit_v_t(2 * g)
                    emit_v_t(2 * g + 1)
                # Phase D: scores h2 + second half of v + y(h0) windows.
                for g in range(4):
                    for tk in range(4 * g, 4 * g + 4):
                        emit_scores_tk(2, tk)
                    emit_v_t(8 + 2 * g)
                    emit_v_t(9 + 2 * g)
                    emit_y_window(0, g)

              # Phase E: scores h3 + y(h1) + y(h2) windows (sps still open).
              for g in range(4):
                  for tk in range(4 * g, 4 * g + 4):
                      emit_scores_tk(3, tk)
                  emit_y_window(1, g)
                  emit_y_window(2, g)

            # ---- projection (sps closed: pps gets its 4 banks) ----
            with (
                tc.tile_pool(name="pps", bufs=4, space="PSUM") as pps,
                tc.tile_pool(name="outs", bufs=8) as outs,
            ):
                def emit_proj_pair(t0):
                    # hp0 halves first: they depend only on earlier heads, so
                    # they hide the y(h3) normalize chain of the current batch.
                    pps_t = {}
                    for t in (t0, t0 + 1):
                        for n in range(2):
                            pp = pps.tile([128, 512], F32, name="pp", tag="p")
                            pps_t[t, n] = pp
                            nc.tensor.matmul(
                                pp[:],
                                yT_all[0][:, 128 * t : 128 * t + 128],
                                wpT_sb[0][:, 512 * n : 512 * n + 512],
                                start=True,
                                stop=False,
                                skip_group_check=True,
                            )
                    for t in (t0, t0 + 1):
                        for n in range(2):
                            pp = pps_t[t, n]
                            nc.tensor.matmul(
                                pp[:],
                                yT_all[1][:, 128 * t : 128 * t + 128],
                                wpT_sb[1][:, 512 * n : 512 * n + 512],
                                start=False,
                                stop=True,
                                skip_group_check=True,
                            )
                            ot = outs.tile([128, 512], BF16, name="ot")
                            # alternate engines: ACT is idle once exp is done
                            if n == 0:
                                nc.vector.tensor_copy(ot[:], pp[:])
                            else:
                                nc.scalar.copy(ot[:], pp[:])
                            # final batch: split across both DMA queues
                            eng = nc.gpsimd if (t >= 14 and n == 1) else nc.sync
                            eng.dma_start(
                                out_d[
                                    128 * t : 128 * t + 128,
                                    512 * n : 512 * n + 512,
                                ],
                                ot[:],
                            )

                # Phase F: y(h3) windows one batch ahead of their
                # projection, so each normalize chain hides under the
                # previous batch's proj matmuls.
                emit_y_window(3, 0)
                emit_y_window(3, 1)
                for j in range(4):
                    emit_proj_pair(4 * j)
                    if j < 2:
                        emit_y_window(3, j + 2)
                    emit_proj_pair(4 * j + 2)

    nc.finalize()
    return nc


def _get_nc():
    global _cached_nc
    if _cached_nc is None:
        _cached_nc = _build()
    return _cached_nc


def kernel(x, Wq, Wk, Wv, Wp, bp):
    global last_results
    x = np.asarray(x, dtype=np.float32)
    Wq = np.asarray(Wq, dtype=np.float32)
    Wk = np.asarray(Wk, dtype=np.float32)
    Wv = np.asarray(Wv, dtype=np.float32)
    Wp = np.asarray(Wp, dtype=np.float32)
    bp = np.asarray(bp, dtype=np.float32)

    WpT = np.ascontiguousarray(Wp.T)  # [C_in(features), C_out]
    mask01 = np.triu(np.ones((128, 128), dtype=np.float32)).astype(ml_dtypes.bfloat16)

    def chunked(w):
        # [C, m] -> [128, 8*m]: c-chunk c at cols [m*c : m*(c+1)]
        m = w.shape[1]
        return np.ascontiguousarray(
            w.reshape(8, 128, m).transpose(1, 0, 2).reshape(128, 8 * m)
        ).astype(ml_dtypes.bfloat16)

    xT_by_batch = [
        np.ascontiguousarray(x[b].T).astype(ml_dtypes.bfloat16) for b in range(B)
    ]
    in_maps = []
    for core in range(N_CORES):
        b, g = core // 4, core % 4
        h0 = HPC * g
        wq_p = np.stack(
            [chunked(np.concatenate([Wq[h0 + 2 * hp], Wq[h0 + 2 * hp + 1]], axis=1))
             for hp in range(2)]
        )  # [2, 128, 1024] bf16
        wk_p = np.stack(
            [chunked(np.concatenate([Wk[h0 + 2 * hp], Wk[h0 + 2 * hp + 1]], axis=1))
             for hp in range(2)]
        )
        wv_p = chunked(
            np.concatenate([Wv[h0 + j] for j in range(HPC)], axis=1)
        )  # [128, 2048] bf16
        wpT_p = np.ascontiguousarray(
            WpT[256 * g : 256 * (g + 1)].reshape(2, 128, C)
        ).astype(ml_dtypes.bfloat16)
        in_maps.append(
            {
                "xT": xT_by_batch[b],
                "wq": wq_p,
                "wk": wk_p,
                "wv": wv_p,
                "wpT": wpT_p,
                "mask": mask01,
            }
        )

    nc = _get_nc()
    kwargs = {}
    if os.environ.get("KERNEL_TRACE", "0") == "1":
        kwargs = dict(trace=True, trace_cores=list(range(N_CORES)),
                      stitch_traces=True)
    try:
        res = run_bass_kernel_spmd(
            nc, in_maps, core_ids=list(range(N_CORES)), **kwargs
        )
    except ModuleNotFoundError:
        # tracing unavailable in this environment; run untraced
        res = run_bass_kernel_spmd(nc, in_maps, core_ids=list(range(N_CORES)))
    last_results = res

    out = np.zeros((B, T, C), dtype=np.float32)
    for core in range(N_CORES):
        b = core // 4
        out[b] += res.results[core]["out"].astype(np.float32)
    out += bp[None, None, :]
    return out



# revision 22
# speedup vs baseline: 1.0175x; 1.0175x over previous
"""Multi-head causal attention (B=2, T=2048, C=1024, H=16, S=64) on 8 TRN2 cores.

Sharding: core i handles batch b = i//4 and head group g = i%4 (4 heads each).
Each core computes a partial output projection (its heads' contribution to the
full [T, C] output); the host sums the 4 partials per batch and adds the bias.

V2 dataflow (cost model: matmul time = out_free_cols x cycles(moving dtype),
fp8e4 DoubleRow = 0.5 cycles/col):
  - QKV projections: error-compensated fp8 (x = xh+xl, W*32 = wh+wl; terms
    xh*wh + xh*wl + xl*wh via DoubleRow chunk pairs; xl*wl dropped).
    12 DR matmuls replace 8 bf16 matmuls per tile: 25% fewer PE cycles.
  - Scores: q,k quantized to fp8; off-diagonal tiles via zero-padded
    DoubleRow (stationary = (k_tile, zeros), moving = q twice, stride-0) at
    0.5 cycles/col. Diagonal tiles in bf16 for accuracy (softmax spike).
  - p = exp(s * 0.125/1024) on ACT (weights carry x32 scale per side).
  - AV reoriented: stationary = p tile [u,tq], moving = v|1 [u,65]; out
    y[tq, 65] accumulated over u tiles in PSUM (col 64 = denominator).
  - Normalize y by 1/d per (pair, tq-tile) on DVE (stride-0 broadcast mul),
    transpose y -> yT via PE matmul against identity.
  - Output projection bf16 (stationary yT, moving WpT), accumulate head
    pairs in PSUM.
"""

import os
import math
import numpy as np
import ml_dtypes

import concourse.bacc as bacc
import concourse.mybir as mybir
import concourse.tile as tile
from concourse.bass_utils import run_bass_kernel_spmd

F32 = mybir.dt.float32
BF16 = mybir.dt.bfloat16
E4 = mybir.dt.float8e4

B, T, C, H, S = 2, 2048, 1024, 16, 64
HPC = 4          # heads per core
N_CORES = 8
NC_T = T // 128  # 16 t-tiles of 128
WS = 32.0        # weight scale folded into fp8 weights

# p storage offsets: tile tk spans tq in [128*tk, 2048)
SPAN = [T - 128 * tk for tk in range(NC_T)]
OFF = [0] * NC_T
for _tk in range(1, NC_T):
    OFF[_tk] = OFF[_tk - 1] + SPAN[_tk - 1]
ATT_W = OFF[-1] + SPAN[-1]  # 17408

_cached_nc = None
last_results = None  # BassKernelResults of the most recent run (for test harness)


def _build():
    nc = bacc.Bacc("TRN2", target_bir_lowering=False)

    # fp8 hi/lo inputs, chunk-major so each DMA is contiguous per partition.
    xh_d = nc.dram_tensor("xh", [128, 8 * 2048], E4, kind="ExternalInput")
    xl_d = nc.dram_tensor("xl", [128, 8 * 2048], E4, kind="ExternalInput")
    wq_d = nc.dram_tensor("wq", [2, 128, 2 * 8 * 128], E4, kind="ExternalInput")
    wk_d = nc.dram_tensor("wk", [2, 128, 2 * 8 * 128], E4, kind="ExternalInput")
    wv_d = nc.dram_tensor("wv", [128, 2 * 8 * 256], E4, kind="ExternalInput")
    wpT_d = nc.dram_tensor("wpT", [2, 128, C], BF16, kind="ExternalInput")
    mask_d = nc.dram_tensor("mask", [128, 128], BF16, kind="ExternalInput")
    ident_d = nc.dram_tensor("ident", [128, 128], BF16, kind="ExternalInput")
    out_d = nc.dram_tensor("out", [T, C], BF16, kind="ExternalOutput")

    with tile.TileContext(nc) as tc:
        with (
            tc.tile_pool(name="const", bufs=1) as constp,
            tc.tile_pool(name="qk8", bufs=1) as qk8p,
            tc.tile_pool(name="qk16", bufs=1) as qk16p,
            tc.tile_pool(name="vsb", bufs=1) as vp,
            tc.tile_pool(name="ysb", bufs=1) as ysbp,
            tc.tile_pool(name="yT", bufs=1) as ytcp,
            tc.tile_pool(name="attp", bufs=1) as attp,
            tc.tile_pool(name="sm", bufs=2) as smp,
            tc.tile_pool(name="ypsum", bufs=2, space="PSUM") as yps,
        ):
            # persistent tiles
            mask_sb = constp.tile([128, 128], BF16, name="mask_sb")
            nc.gpsimd.dma_start(mask_sb[:], mask_d[:])
            ident_sb = constp.tile([128, 128], BF16, name="ident_sb")
            nc.gpsimd.dma_start(ident_sb[:], ident_d[:])

            # fp8 q/k: q gets 128 zero-pad cols (moving side of the
            # zero-padded DoubleRow score matmuls; q writes complete early
            # so the wide moving AP causes no late false deps)
            q8 = [qk8p.tile([128, T + 128], E4, name=f"q8_{hp}") for hp in range(2)]
            k8 = [qk8p.tile([128, T], E4, name=f"k8_{hp}") for hp in range(2)]
            for hp in range(2):
                nc.vector.memset(q8[hp][:, T : T + 128], 0.0)
            # bf16 q/k for the diagonal score tiles
            q16 = [qk16p.tile([128, T], BF16, name=f"q16_{hp}") for hp in range(2)]
            k16 = [qk16p.tile([128, T], BF16, name=f"k16_{hp}") for hp in range(2)]

            # v tiles: [128, 4*65] bf16; head h in cols 65h..65h+63, col 65h+64 = 1
            v_sb = [vp.tile([128, 4 * 65], BF16, name=f"v{t}") for t in range(NC_T)]
            for t in range(NC_T):
                ones_ap = v_sb[t].rearrange("p (h c) -> p h c", h=4)[:, :, 64]
                nc.vector.memset(ones_ap, 1.0)

            # normalized y staging [tq, (h_even|h_odd)] per pair, per tq tile
            y_sb = [ysbp.tile([128, NC_T * 128], BF16, name=f"ysb{hp}")
                    for hp in range(2)]
            # concatenated yT for proj: dim1 = hp
            yT_cat = ytcp.tile([128, 2 * T], BF16, name="yT_cat")

            # p buffers (3-deep head pipeline: exp(h+2) must not wait on
            # AV(h) finishing)
            att_buf = [attp.tile([128, ATT_W], BF16, name=f"attb{i}")
                       for i in range(3)]
            BUF_OF = [0, 1, 2, 0]  # head -> p buffer

            wpT_sb = [constp.tile([128, C], BF16, name=f"wpT{hp}")
                      for hp in range(2)]

            def emit_scores_tk(h, tk, sps_pool, part_w=1024):
                """Scores for head h, k-tile tk: diag 128 cols in bf16 +
                off-diag in zero-padded fp8 DR chunks; exp into att_buf."""
                hp, half = h // 2, h % 2
                r0 = 64 * half
                ab = att_buf[BUF_OF[h]]
                span = SPAN[tk]
                kt16 = k16[hp][r0 : r0 + 64, 128 * tk : 128 * tk + 128]
                # stationary fp8: (k tile, k tile) via stride-0 broadcast;
                # the moving q side supplies (q cols, zeros) so the second
                # k contribution is k.T @ 0 = 0.
                kt8 = (
                    k8[hp][r0 : r0 + 64, 128 * tk : 128 * tk + 128]
                    .unsqueeze(1)
                    .broadcast_to([64, 2, 128])
                )
                for part in range(math.ceil(span / part_w)):
                    pspan = min(part_w, span - part_w * part)
                    pt = sps_pool.tile([128, part_w], F32, name="sps_t", tag="s")
                    c0 = 0
                    if part == 0:
                        # diagonal block in bf16 (K=64)
                        nc.tensor.matmul(
                            pt[:, 0:128],
                            kt16,
                            q16[hp][r0 : r0 + 64, 128 * tk : 128 * tk + 128],
                            start=True,
                            stop=True,
                        )
                        c0 = 128
                    while c0 < pspan:
                        n = min(128, pspan - c0)
                        tq0 = 128 * tk + part_w * part + c0
                        # moving: (q cols tq0.., zero pad at col T), built by
                        # 128-col rechunking + stride slicing
                        nch = (T + 128 - tq0) // 128
                        qmov = (
                            q8[hp][r0 : r0 + 64, tq0 : T + 128]
                            .rearrange("p (x m) -> p x m", x=nch)[:, 0 :: max(nch - 1, 1), :]
                        )
                        if nch == 1:
                            qmov = qmov.broadcast_to([64, 2, n])
                        nc.tensor.matmul(
                            pt[:, c0 : c0 + n],
                            kt8,
                            qmov[:, :, 0:n] if n != 128 else qmov,
                            start=True,
                            stop=True,
                            perf_mode=mybir.MatmulPerfMode.DoubleRow,
                        )
                        c0 += n
                    dst = ab[
                        :, OFF[tk] + part_w * part : OFF[tk] + part_w * part + pspan
                    ]
                    nc.scalar.activation(
                        dst,
                        pt[:, 0:pspan],
                        mybir.ActivationFunctionType.Exp,
                        scale=0.125 / (WS * WS),
                    )
                # mask the diagonal block (first 128 cols of this tk tile)
                diag = ab[:, OFF[tk] : OFF[tk] + 128]
                nc.gpsimd.tensor_mul(diag, diag, mask_sb[:])

            def emit_av_pair(hp, j):
                """y[tq, 65] for both heads of pair hp, tq-tile j, then
                normalize into y_sb[hp] block j."""
                yp = yps.tile([128, 512], F32, name="yps_t", tag="y")
                for half in range(2):
                    h = 2 * hp + half
                    ab = att_buf[BUF_OF[h]]
                    for tk in range(j + 1):
                        ptile = ab[:, OFF[tk] + 128 * (j - tk) : OFF[tk] + 128 * (j - tk) + 128]
                        nc.tensor.matmul(
                            yp[:, 65 * half : 65 * half + 65],
                            ptile,
                            v_sb[tk][:, 65 * h : 65 * h + 65],
                            start=(tk == 0),
                            stop=(tk == j),
                            skip_group_check=True,
                        )
                # normalize: y into y_sb[hp] block j, bf16
                ypv = yp[:, 0:130].rearrange("p (h c) -> p h c", h=2)
                rec = smp.tile([128, 2], F32, name="rec")
                nc.vector.reciprocal(rec[:], ypv[:, :, 64])
                dst = (
                    y_sb[hp][:, 128 * j : 128 * j + 128]
                    .rearrange("p (h c) -> p h c", h=2)
                )
                nc.vector.tensor_mul(
                    dst,
                    ypv[:, :, 0:64],
                    rec[:].unsqueeze(2).broadcast_to([128, 2, 64]),
                )

            def emit_transpose_group(hp, jg, tp_pool, tp_tag):
                """Transpose y_sb[hp] tiles 4jg..4jg+3 into yT_cat via PE
                matmul with identity; copy PSUM->SBUF bf16 on ACT."""
                tp = tp_pool.tile([128, 512], F32, name="ytp_t", tag=tp_tag)
                for jj in range(4):
                    j = 4 * jg + jj
                    nc.tensor.matmul(
                        tp[:, 128 * jj : 128 * jj + 128],
                        y_sb[hp][:, 128 * j : 128 * j + 128],
                        ident_sb[:],
                        start=True,
                        stop=True,
                        skip_group_check=True,
                    )
                nc.scalar.copy(
                    yT_cat[:, T * hp + 512 * jg : T * hp + 512 * jg + 512], tp[:]
                )

            # ---- scores/QKV scope ----
            with (
                tc.tile_pool(name="sps", bufs=2, space="PSUM") as sps,
                tc.tile_pool(name="xw", bufs=1) as xw,
                tc.tile_pool(name="mmps", bufs=2, space="PSUM") as mmps,
            ):
                xh_sb = xw.tile([128, 8 * 2048], E4, name="xh")
                xl_sb = xw.tile([128, 8 * 2048], E4, name="xl")
                wq_sb = [xw.tile([128, 2 * 8 * 128], E4, name=f"wq{hp}")
                         for hp in range(2)]
                wk_sb = [xw.tile([128, 2 * 8 * 128], E4, name=f"wk{hp}")
                         for hp in range(2)]
                wv_sb = xw.tile([128, 2 * 8 * 256], E4, name="wv")

                # x + weights on SP HWDGE and SWDGE only: the ACT queue must
                # stay clear, DMA dispatches there would serialize with exp
                # on the ACT sequencer.
                xh_r = xh_sb.rearrange("p (c n) -> p c n", c=8)
                xl_r = xl_sb.rearrange("p (c n) -> p c n", c=8)
                xhd_r = xh_d[:].rearrange("p (c n) -> p c n", c=8)
                xld_r = xl_d[:].rearrange("p (c n) -> p c n", c=8)
                # all of x on SP in consumption order, few big transfers
                # (each DMACopy pays ~625ns HWDGE generation serially)
                nc.sync.dma_start(wq_sb[0][:], wq_d[0])
                nc.sync.dma_start(xh_sb[:, 0 : 4 * 2048], xh_d[:, 0 : 4 * 2048])
                nc.sync.dma_start(xh_sb[:, 4 * 2048 :], xh_d[:, 4 * 2048 :])
                nc.sync.dma_start(wk_sb[0][:], wk_d[0])
                nc.sync.dma_start(xl_sb[:, 0 : 4 * 2048], xl_d[:, 0 : 4 * 2048])
                nc.sync.dma_start(xl_sb[:, 4 * 2048 :], xl_d[:, 4 * 2048 :])
                nc.sync.dma_start(wq_sb[1][:], wq_d[1])
                nc.sync.dma_start(wk_sb[1][:], wk_d[1])
                nc.sync.dma_start(wv_sb[:], wv_d[:])

                def emit_qk_group(hp, kind, tq):
                    """q or k for head pair hp, 512 t-cols starting 512*tq.
                    Compensated fp8: xh*wh + xh*wl + xl*wh, DR chunk pairs.
                    One DVE fp8 copy + one Pool bf16 copy per group."""
                    w_sb = wq_sb if kind == 0 else wk_sb
                    d8 = q8[hp] if kind == 0 else k8[hp]
                    d16 = q16[hp] if kind == 0 else k16[hp]
                    pt = mmps.tile([128, 512], F32, name="qkps", tag="qk")
                    for half in range(2):
                        n0 = 512 * tq + 256 * half
                        first = True
                        for xs, wl_ in ((xh_r, 0), (xh_r, 1), (xl_r, 0)):
                            wr = w_sb[hp][:, 1024 * wl_ : 1024 * wl_ + 1024].rearrange(
                                "p (c m) -> p c m", c=8)
                            for cp in range(4):
                                nc.tensor.matmul(
                                    pt[:, 256 * half : 256 * half + 256],
                                    wr[:, 2 * cp : 2 * cp + 2, :],
                                    xs[:, 2 * cp : 2 * cp + 2, n0 : n0 + 256],
                                    start=first,
                                    stop=(wl_ == 0 and xs is xl_r and cp == 3),
                                    perf_mode=mybir.MatmulPerfMode.DoubleRow,
                                )
                                first = False
                    n0 = 512 * tq
                    # Pool cannot read PSUM: bf16 from PSUM on DVE, then
                    # fp8 from the bf16 copy on Pool (SBUF->SBUF)
                    nc.vector.tensor_copy(d16[:, n0 : n0 + 512], pt[:])
                    nc.gpsimd.tensor_copy(
                        d8[:, n0 : n0 + 512], d16[:, n0 : n0 + 512]
                    )

                def emit_v_t(t):
                    pv = mmps.tile([128, 512], F32, name="vps", tag="qk")[:, 0:256]
                    first = True
                    for xs, wl_ in ((xh_r, 0), (xh_r, 1), (xl_r, 0)):
                        wr = wv_sb[:, 2048 * wl_ : 2048 * wl_ + 2048].rearrange(
                            "p (c m) -> p c m", c=8)
                        for cp in range(4):
                            nc.tensor.matmul(
                                pv[:],
                                xs[:, 2 * cp : 2 * cp + 2, 128 * t : 128 * t + 128],
                                wr[:, 2 * cp : 2 * cp + 2, :],
                                start=first,
                                stop=(wl_ == 0 and xs is xl_r and cp == 3),
                                perf_mode=mybir.MatmulPerfMode.DoubleRow,
                            )
                            first = False
                    # v = pv / WS, bf16
                    nc.vector.tensor_scalar_mul(
                        v_sb[t].rearrange("p (h c) -> p h c", h=4)[:, :, 0:64],
                        pv[:].rearrange("p (h c) -> p h c", h=4),
                        1.0 / WS,
                    )

                # PE warm-up on the mask tile while input DMAs land.
                warm = sps.tile([128, 1024], F32, name="warm", tag="s")
                for i in range(16):
                    nc.tensor.matmul(
                        warm[:, 0:128], mask_sb[:], mask_sb[:],
                        start=True, stop=True,
                    )
                # Phase A: q(hp0)
                for tq in range(4):
                    emit_qk_group(0, 0, tq)
                for hp in range(2):
                    nc.gpsimd.dma_start(wpT_sb[hp][:], wpT_d[hp])
                # Phase B: k(hp0) + scores h0 + q(hp1), finely interleaved
                # so ACT gets a steady exp supply.
                for g in range(4):
                    emit_qk_group(0, 1, g)
                    emit_scores_tk(0, 4 * g, sps)
                    emit_scores_tk(0, 4 * g + 1, sps)
                    emit_qk_group(1, 0, g)
                    emit_scores_tk(0, 4 * g + 2, sps)
                    emit_scores_tk(0, 4 * g + 3, sps)
                # Phase C: k(hp1) + scores h1 + v
                for g in range(4):
                    emit_qk_group(1, 1, g)
                    emit_scores_tk(1, 4 * g, sps)
                    emit_v_t(4 * g)
                    emit_scores_tk(1, 4 * g + 1, sps)
                    emit_v_t(4 * g + 1)
                    emit_scores_tk(1, 4 * g + 2, sps)
                    emit_v_t(4 * g + 2)
                    emit_scores_tk(1, 4 * g + 3, sps)
                    emit_v_t(4 * g + 3)
                # Phase D: scores h2 + AV pair hp0 + transposes hp0
                for g in range(4):
                    for i in range(4):
                        emit_scores_tk(2, 4 * g + i, sps)
                        emit_av_pair(0, 4 * g + i)
                    emit_transpose_group(0, g, mmps, "qk")

            # ---- tail scope: scores h3 + AV hp1 + proj ----
            with (
                tc.tile_pool(name="sps2", bufs=2, space="PSUM") as sps2,
                tc.tile_pool(name="pps", bufs=2, space="PSUM") as pps,
                tc.tile_pool(name="outs", bufs=8) as outs,
            ):
                def emit_proj_t(t):
                    pps_t = {}
                    for n in range(2):
                        pp = pps.tile([128, 512], F32, name="pp", tag="p")
                        pps_t[n] = pp
                        nc.tensor.matmul(
                            pp[:],
                            yT_cat[:, 128 * t : 128 * t + 128],
                            wpT_sb[0][:, 512 * n : 512 * n + 512],
                            start=True,
                            stop=False,
                            skip_group_check=True,
                        )
                    for n in range(2):
                        pp = pps_t[n]
                        nc.tensor.matmul(
                            pp[:],
                            yT_cat[:, T + 128 * t : T + 128 * t + 128],
                            wpT_sb[1][:, 512 * n : 512 * n + 512],
                            start=False,
                            stop=True,
                            skip_group_check=True,
                        )
                        ot = outs.tile([128, 512], BF16, name="ot")
                        if n == 0:
                            nc.vector.tensor_copy(ot[:], pp[:])
                        else:
                            nc.scalar.copy(ot[:], pp[:])
                        eng = nc.gpsimd if (t >= 14 and n == 1) else nc.sync
                        eng.dma_start(
                            out_d[
                                128 * t : 128 * t + 128,
                                512 * n : 512 * n + 512,
                            ],
                            ot[:],
                        )

                # Phase E: scores h3 (512-col psum chunks) + AV hp1 +
                # transposes hp1 + early proj pairs
                for g in range(4):
                    for i in range(4):
                        emit_scores_tk(3, 4 * g + i, sps2, part_w=1024)
                        emit_av_pair(1, 4 * g + i)
                        if g >= 1:
                            emit_proj_t(4 * (g - 1) + i)
                    emit_transpose_group(1, g, yps, "y")
                # Phase F: remaining proj
                for t in range(12, 16):
                    emit_proj_t(t)

    nc.finalize()
    return nc


def _get_nc():
    global _cached_nc
    if _cached_nc is None:
        _cached_nc = _build()
    return _cached_nc


def kernel(x, Wq, Wk, Wv, Wp, bp):
    global last_results
    x = np.asarray(x, dtype=np.float32)
    Wq = np.asarray(Wq, dtype=np.float32)
    Wk = np.asarray(Wk, dtype=np.float32)
    Wv = np.asarray(Wv, dtype=np.float32)
    Wp = np.asarray(Wp, dtype=np.float32)
    bp = np.asarray(bp, dtype=np.float32)

    E4NP = ml_dtypes.float8_e4m3
    WpT = np.ascontiguousarray(Wp.T)  # [C_in(features), C_out]
    mask01 = np.triu(np.ones((128, 128), dtype=np.float32)).astype(ml_dtypes.bfloat16)
    ident = np.eye(128, dtype=np.float32).astype(ml_dtypes.bfloat16)

    def chunked(w):
        # [C, m] -> [128, 8*m]: c-chunk c at cols [m*c : m*(c+1)]
        m = w.shape[1]
        return np.ascontiguousarray(
            w.reshape(8, 128, m).transpose(1, 0, 2).reshape(128, 8 * m)
        )

    def hilo(a):
        hi = a.astype(E4NP)
        lo = (a - hi.astype(np.float32)).astype(E4NP)
        return hi, lo

    xT_by_batch = [np.ascontiguousarray(x[b].T) for b in range(B)]
    xhl_by_batch = [hilo(chunked(xT_by_batch[b])) for b in range(B)]

    in_maps = []
    for core in range(N_CORES):
        b, g = core // 4, core % 4
        h0 = HPC * g
        def wq_pair(W):
            res = []
            for hp in range(2):
                cat = np.concatenate(
                    [W[h0 + 2 * hp], W[h0 + 2 * hp + 1]], axis=1
                ) * WS
                h_, l_ = hilo(chunked(cat))
                res.append(np.concatenate([h_, l_], axis=1))
            return np.stack(res)
        wq_p = wq_pair(Wq)
        wk_p = wq_pair(Wk)
        wv_cat = np.concatenate([Wv[h0 + j] for j in range(HPC)], axis=1) * WS
        wvh, wvl = hilo(chunked(wv_cat))
        wv_p = np.concatenate([wvh, wvl], axis=1)
        wpT_p = np.ascontiguousarray(
            WpT[256 * g : 256 * (g + 1)].reshape(2, 128, C)
        ).astype(ml_dtypes.bfloat16)
        xh, xl = xhl_by_batch[b]
        in_maps.append(
            {
                "xh": xh, "xl": xl,
                "wq": wq_p, "wk": wk_p, "wv": wv_p,
                "wpT": wpT_p,
                "mask": mask01,
                "ident": ident,
            }
        )

    nc = _get_nc()
    kwargs = {}
    if os.environ.get("KERNEL_TRACE", "0") == "1":
        kwargs = dict(trace=True, trace_cores=list(range(N_CORES)),
                      stitch_traces=True)
    try:
        res = run_bass_kernel_spmd(
            nc, in_maps, core_ids=list(range(N_CORES)), **kwargs
        )
    except ModuleNotFoundError:
        res = run_bass_kernel_spmd(nc, in_maps, core_ids=list(range(N_CORES)))
    last_results = res

    out = np.zeros((B, T, C), dtype=np.float32)
    for core in range(N_CORES):
        b = core // 4
        out[b] += res.results[core]["out"].astype(np.float32)
    out += bp[None, None, :]
    return out


# revision 24
# speedup vs baseline: 1.0651x; 1.0468x over previous
"""Multi-head causal attention (B=2, T=2048, C=1024, H=16, S=64) on 8 TRN2 cores.

Sharding: core i handles batch b = i//4 and head group g = i%4 (4 heads each).
Each core computes a partial output projection (its heads' contribution to the
full [T, C] output); the host sums the 4 partials per batch and adds the bias.

V2 dataflow (cost model: matmul time = out_free_cols x cycles(moving dtype),
fp8e4 DoubleRow = 0.5 cycles/col):
  - QKV projections: error-compensated fp8 (x = xh+xl, W*32 = wh+wl; terms
    xh*wh + xh*wl + xl*wh via DoubleRow chunk pairs; xl*wl dropped).
    12 DR matmuls replace 8 bf16 matmuls per tile: 25% fewer PE cycles.
  - Scores: q,k quantized to fp8; off-diagonal tiles via zero-padded
    DoubleRow (stationary = (k_tile, zeros), moving = q twice, stride-0) at
    0.5 cycles/col. Diagonal tiles in bf16 for accuracy (softmax spike).
  - p = exp(s * 0.125/1024) on ACT (weights carry x32 scale per side).
  - AV reoriented: stationary = p tile [u,tq], moving = v|1 [u,65]; out
    y[tq, 65] accumulated over u tiles in PSUM (col 64 = denominator).
  - Normalize y by 1/d per (pair, tq-tile) on DVE (stride-0 broadcast mul),
    transpose y -> yT via PE matmul against identity.
  - Output projection bf16 (stationary yT, moving WpT), accumulate head
    pairs in PSUM.
"""

import os
import math
import numpy as np
import ml_dtypes

import concourse.bacc as bacc
import concourse.mybir as mybir
import concourse.tile as tile
from concourse.bass_utils import run_bass_kernel_spmd

F32 = mybir.dt.float32
BF16 = mybir.dt.bfloat16
E4 = mybir.dt.float8e4

B, T, C, H, S = 2, 2048, 1024, 16, 64
HPC = 4          # heads per core
N_CORES = 8
NC_T = T // 128  # 16 t-tiles of 128
WS = 32.0        # weight scale folded into fp8 weights

# p storage offsets: tile tk spans tq in [128*tk, 2048)
SPAN = [T - 128 * tk for tk in range(NC_T)]
OFF = [0] * NC_T
for _tk in range(1, NC_T):
    OFF[_tk] = OFF[_tk - 1] + SPAN[_tk - 1]
ATT_W = OFF[-1] + SPAN[-1]  # 17408

_cached_nc = None
last_results = None  # BassKernelResults of the most recent run (for test harness)


def _build():
    nc = bacc.Bacc("TRN2", target_bir_lowering=False)

    # fp8 hi/lo inputs, chunk-major so each DMA is contiguous per partition.
    xh_d = nc.dram_tensor("xh", [128, 8 * 2048], E4, kind="ExternalInput")
    xl_d = nc.dram_tensor("xl", [128, 8 * 2048], E4, kind="ExternalInput")
    wq_d = nc.dram_tensor("wq", [2, 128, 2 * 8 * 128], E4, kind="ExternalInput")
    wk_d = nc.dram_tensor("wk", [2, 128, 2 * 8 * 128], E4, kind="ExternalInput")
    wv_d = nc.dram_tensor("wv", [128, 2 * 8 * 256], E4, kind="ExternalInput")
    wpT_d = nc.dram_tensor("wpT", [2, 128, C], BF16, kind="ExternalInput")
    mask_d = nc.dram_tensor("mask", [128, 128], BF16, kind="ExternalInput")
    ident_d = nc.dram_tensor("ident", [128, 128], BF16, kind="ExternalInput")
    out_d = nc.dram_tensor("out", [T, C], BF16, kind="ExternalOutput")

    with tile.TileContext(nc) as tc:
        with (
            tc.tile_pool(name="const", bufs=1) as constp,
            tc.tile_pool(name="qk8", bufs=1) as qk8p,
            tc.tile_pool(name="qk16", bufs=1) as qk16p,
            tc.tile_pool(name="vsb", bufs=1) as vp,
            tc.tile_pool(name="ysb", bufs=1) as ysbp,
            tc.tile_pool(name="yT", bufs=1) as ytcp,
            tc.tile_pool(name="attp", bufs=1) as attp,
            tc.tile_pool(name="sm", bufs=2) as smp,
            tc.tile_pool(name="ypsum", bufs=2, space="PSUM") as yps,
        ):
            # persistent tiles
            mask_sb = constp.tile([128, 128], BF16, name="mask_sb")
            nc.gpsimd.dma_start(mask_sb[:], mask_d[:])
            ident_sb = constp.tile([128, 128], BF16, name="ident_sb")
            nc.gpsimd.dma_start(ident_sb[:], ident_d[:])

            # fp8 q/k: q gets 128 zero-pad cols (moving side of the
            # zero-padded DoubleRow score matmuls; q writes complete early
            # so the wide moving AP causes no late false deps)
            q8 = [qk8p.tile([128, T + 128], E4, name=f"q8_{hp}") for hp in range(2)]
            k8 = [qk8p.tile([128, T], E4, name=f"k8_{hp}") for hp in range(2)]
            for hp in range(2):
                nc.vector.memset(q8[hp][:, T : T + 128], 0.0)
            # bf16 q/k for the diagonal score tiles
            q16 = [qk16p.tile([128, T], BF16, name=f"q16_{hp}") for hp in range(2)]
            k16 = [qk16p.tile([128, T], BF16, name=f"k16_{hp}") for hp in range(2)]

            # v tiles: [128, 4*65] bf16; head h in cols 65h..65h+63, col 65h+64 = 1
            v_sb = [vp.tile([128, 4 * 65], BF16, name=f"v{t}") for t in range(NC_T)]
            for t in range(NC_T):
                ones_ap = v_sb[t].rearrange("p (h c) -> p h c", h=4)[:, :, 64]
                nc.vector.memset(ones_ap, 1.0)

            # normalized y staging [tq, (h_even|h_odd)] per pair, per tq tile
            y_sb = [ysbp.tile([128, NC_T * 128], BF16, name=f"ysb{hp}")
                    for hp in range(2)]
            # concatenated yT for proj: dim1 = hp
            yT_cat = ytcp.tile([128, 2 * T], BF16, name="yT_cat")

            # p buffers (3-deep head pipeline: exp(h+2) must not wait on
            # AV(h) finishing)
            att_buf = [attp.tile([128, ATT_W], BF16, name=f"attb{i}")
                       for i in range(3)]
            BUF_OF = [0, 1, 2, 0]  # head -> p buffer

            wpT_sb = [constp.tile([128, C], BF16, name=f"wpT{hp}")
                      for hp in range(2)]

            # Schraudolph fast-exp constants (bf16 bit trick):
            # bits_i16 = A*z + B with z = s_psum * 0.125/WS^2
            SCH_A = 184.66496 * 0.125 / (WS * WS)
            SCH_B = 16252.0
            exp_ctr = [0]

            def emit_exp(dst, src):
                exp_ctr[0] += 1
                if exp_ctr[0] % 8 == 0:
                    # DVE fast-exp: affine into int16, bitcast to bf16
                    nc.vector.tensor_scalar(
                        dst.bitcast(mybir.dt.int16),
                        src,
                        SCH_A,
                        SCH_B,
                        mybir.AluOpType.mult,
                        mybir.AluOpType.add,
                    )
                else:
                    nc.scalar.activation(
                        dst, src,
                        mybir.ActivationFunctionType.Exp,
                        scale=0.125 / (WS * WS),
                    )

            def emit_scores_tk(h, tk, sps_pool, part_w=1024, only_part=None):
                """Scores for head h, k-tile tk: diag 128 cols in bf16 +
                off-diag in zero-padded fp8 DR chunks; exp into att_buf."""
                hp, half = h // 2, h % 2
                r0 = 64 * half
                ab = att_buf[BUF_OF[h]]
                span = SPAN[tk]
                kt16 = k16[hp][r0 : r0 + 64, 128 * tk : 128 * tk + 128]
                # stationary fp8: (k tile, k tile) via stride-0 broadcast;
                # the moving q side supplies (q cols, zeros) so the second
                # k contribution is k.T @ 0 = 0.
                kt8 = (
                    k8[hp][r0 : r0 + 64, 128 * tk : 128 * tk + 128]
                    .unsqueeze(1)
                    .broadcast_to([64, 2, 128])
                )
                for part in range(math.ceil(span / part_w)):
                    if only_part is not None and part != only_part:
                        continue
                    pspan = min(part_w, span - part_w * part)
                    pt = sps_pool.tile([128, part_w], F32, name="sps_t", tag="s")
                    c0 = 0
                    if part == 0:
                        # diagonal block in bf16 (K=64)
                        nc.tensor.matmul(
                            pt[:, 0:128],
                            kt16,
                            q16[hp][r0 : r0 + 64, 128 * tk : 128 * tk + 128],
                            start=True,
                            stop=True,
                        )
                        c0 = 128
                    while c0 < pspan:
                        n = min(128, pspan - c0)
                        tq0 = 128 * tk + part_w * part + c0
                        # moving: (q cols tq0.., zero pad at col T), built by
                        # 128-col rechunking + stride slicing
                        nch = (T + 128 - tq0) // 128
                        qmov = (
                            q8[hp][r0 : r0 + 64, tq0 : T + 128]
                            .rearrange("p (x m) -> p x m", x=nch)[:, 0 :: max(nch - 1, 1), :]
                        )
                        if nch == 1:
                            qmov = qmov.broadcast_to([64, 2, n])
                        nc.tensor.matmul(
                            pt[:, c0 : c0 + n],
                            kt8,
                            qmov[:, :, 0:n] if n != 128 else qmov,
                            start=True,
                            stop=True,
                            perf_mode=mybir.MatmulPerfMode.DoubleRow,
                        )
                        c0 += n
                    dst = ab[
                        :, OFF[tk] + part_w * part : OFF[tk] + part_w * part + pspan
                    ]
                    emit_exp(dst, pt[:, 0:pspan])
                # mask the diagonal block (first 128 cols of this tk tile)
                diag = ab[:, OFF[tk] : OFF[tk] + 128]
                nc.gpsimd.tensor_mul(diag, diag, mask_sb[:])

            def emit_av_pair(hp, j):
                """y[tq, 65] for both heads of pair hp, tq-tile j, then
                normalize into y_sb[hp] block j."""
                yp = yps.tile([128, 512], F32, name="yps_t", tag="y")
                for half in range(2):
                    h = 2 * hp + half
                    ab = att_buf[BUF_OF[h]]
                    for tk in range(j + 1):
                        ptile = ab[:, OFF[tk] + 128 * (j - tk) : OFF[tk] + 128 * (j - tk) + 128]
                        nc.tensor.matmul(
                            yp[:, 65 * half : 65 * half + 65],
                            ptile,
                            v_sb[tk][:, 65 * h : 65 * h + 65],
                            start=(tk == 0),
                            stop=(tk == j),
                            skip_group_check=True,
                        )
                # normalize: y into y_sb[hp] block j, bf16
                ypv = yp[:, 0:130].rearrange("p (h c) -> p h c", h=2)
                rec = smp.tile([128, 2], F32, name="rec")
                nc.vector.reciprocal(rec[:], ypv[:, :, 64])
                dst = (
                    y_sb[hp][:, 128 * j : 128 * j + 128]
                    .rearrange("p (h c) -> p h c", h=2)
                )
                nc.vector.tensor_mul(
                    dst,
                    ypv[:, :, 0:64],
                    rec[:].unsqueeze(2).broadcast_to([128, 2, 64]),
                )

            def emit_transpose_group(hp, jg, tp_pool, tp_tag):
                """Transpose y_sb[hp] tiles 4jg..4jg+3 into yT_cat via PE
                matmul with identity; copy PSUM->SBUF bf16 on ACT."""
                tp = tp_pool.tile([128, 512], F32, name="ytp_t", tag=tp_tag)
                for jj in range(4):
                    j = 4 * jg + jj
                    nc.tensor.matmul(
                        tp[:, 128 * jj : 128 * jj + 128],
                        y_sb[hp][:, 128 * j : 128 * j + 128],
                        ident_sb[:],
                        start=True,
                        stop=True,
                        skip_group_check=True,
                    )
                nc.vector.tensor_copy(
                    yT_cat[:, T * hp + 512 * jg : T * hp + 512 * jg + 512], tp[:]
                )

            # ---- scores/QKV scope ----
            with (
                tc.tile_pool(name="sps", bufs=2, space="PSUM") as sps,
                tc.tile_pool(name="xw", bufs=1) as xw,
                tc.tile_pool(name="mmps", bufs=2, space="PSUM") as mmps,
            ):
                xh_sb = xw.tile([128, 8 * 2048], E4, name="xh")
                xl_sb = xw.tile([128, 8 * 2048], E4, name="xl")
                wq_sb = [xw.tile([128, 2 * 8 * 128], E4, name=f"wq{hp}")
                         for hp in range(2)]
                wk_sb = [xw.tile([128, 2 * 8 * 128], E4, name=f"wk{hp}")
                         for hp in range(2)]
                wv_sb = xw.tile([128, 2 * 8 * 256], E4, name="wv")

                # x + weights on SP HWDGE and SWDGE only: the ACT queue must
                # stay clear, DMA dispatches there would serialize with exp
                # on the ACT sequencer.
                xh_r = xh_sb.rearrange("p (c n) -> p c n", c=8)
                xl_r = xl_sb.rearrange("p (c n) -> p c n", c=8)
                xhd_r = xh_d[:].rearrange("p (c n) -> p c n", c=8)
                xld_r = xl_d[:].rearrange("p (c n) -> p c n", c=8)
                # all of x on SP in consumption order, few big transfers
                # (each DMACopy pays ~625ns HWDGE generation serially)
                nc.sync.dma_start(wq_sb[0][:], wq_d[0])
                nc.sync.dma_start(xh_sb[:, 0 : 4 * 2048], xh_d[:, 0 : 4 * 2048])
                nc.sync.dma_start(xh_sb[:, 4 * 2048 :], xh_d[:, 4 * 2048 :])
                nc.sync.dma_start(wk_sb[0][:], wk_d[0])
                nc.sync.dma_start(xl_sb[:, 0 : 4 * 2048], xl_d[:, 0 : 4 * 2048])
                nc.sync.dma_start(xl_sb[:, 4 * 2048 :], xl_d[:, 4 * 2048 :])
                nc.sync.dma_start(wq_sb[1][:], wq_d[1])
                nc.sync.dma_start(wk_sb[1][:], wk_d[1])
                nc.sync.dma_start(wv_sb[:], wv_d[:])

                def emit_qk_group(hp, kind, tq):
                    """q or k for head pair hp, 512 t-cols starting 512*tq.
                    Compensated fp8: xh*wh + xh*wl + xl*wh, DR chunk pairs.
                    One DVE fp8 copy + one Pool bf16 copy per group."""
                    w_sb = wq_sb if kind == 0 else wk_sb
                    d8 = q8[hp] if kind == 0 else k8[hp]
                    d16 = q16[hp] if kind == 0 else k16[hp]
                    pt = mmps.tile([128, 512], F32, name="qkps", tag="qk")
                    for half in range(2):
                        n0 = 512 * tq + 256 * half
                        first = True
                        for xs, wl_ in ((xh_r, 0), (xh_r, 1), (xl_r, 0)):
                            wr = w_sb[hp][:, 1024 * wl_ : 1024 * wl_ + 1024].rearrange(
                                "p (c m) -> p c m", c=8)
                            for cp in range(4):
                                nc.tensor.matmul(
                                    pt[:, 256 * half : 256 * half + 256],
                                    wr[:, 2 * cp : 2 * cp + 2, :],
                                    xs[:, 2 * cp : 2 * cp + 2, n0 : n0 + 256],
                                    start=first,
                                    stop=(wl_ == 0 and xs is xl_r and cp == 3),
                                    perf_mode=mybir.MatmulPerfMode.DoubleRow,
                                )
                                first = False
                    n0 = 512 * tq
                    # Pool cannot read PSUM: bf16 from PSUM on DVE, then
                    # fp8 from the bf16 copy on Pool (SBUF->SBUF)
                    nc.vector.tensor_copy(d16[:, n0 : n0 + 512], pt[:])
                    nc.gpsimd.tensor_copy(
                        d8[:, n0 : n0 + 512], d16[:, n0 : n0 + 512]
                    )

                def emit_v_t(t):
                    pv = mmps.tile([128, 512], F32, name="vps", tag="qk")[:, 0:256]
                    first = True
                    for xs, wl_ in ((xh_r, 0), (xh_r, 1), (xl_r, 0)):
                        wr = wv_sb[:, 2048 * wl_ : 2048 * wl_ + 2048].rearrange(
                            "p (c m) -> p c m", c=8)
                        for cp in range(4):
                            nc.tensor.matmul(
                                pv[:],
                                xs[:, 2 * cp : 2 * cp + 2, 128 * t : 128 * t + 128],
                                wr[:, 2 * cp : 2 * cp + 2, :],
                                start=first,
                                stop=(wl_ == 0 and xs is xl_r and cp == 3),
                                perf_mode=mybir.MatmulPerfMode.DoubleRow,
                            )
                            first = False
                    # v = pv / WS, bf16
                    nc.vector.tensor_scalar_mul(
                        v_sb[t].rearrange("p (h c) -> p h c", h=4)[:, :, 0:64],
                        pv[:].rearrange("p (h c) -> p h c", h=4),
                        1.0 / WS,
                    )

                # PE warm-up on the mask tile while input DMAs land.
                warm = sps.tile([128, 1024], F32, name="warm", tag="s")
                for i in range(16):
                    nc.tensor.matmul(
                        warm[:, 0:128], mask_sb[:], mask_sb[:],
                        start=True, stop=True,
                    )
                # Phase A: q(hp0) groups 0,1 then k(hp0) group 0, so the
                # first score parts (tq < 1024) can start before all of q
                # is projected.
                emit_qk_group(0, 0, 0)
                emit_qk_group(0, 0, 1)
                emit_qk_group(0, 1, 0)
                for hp in range(2):
                    nc.gpsimd.dma_start(wpT_sb[hp][:], wpT_d[hp])
                # Phase B head: interleave remaining q groups with the
                # early score parts of tiles 0..3
                emit_scores_tk(0, 0, sps, only_part=0)
                emit_qk_group(0, 0, 2)
                emit_scores_tk(0, 1, sps, only_part=0)
                emit_qk_group(0, 0, 3)
                emit_scores_tk(0, 2, sps, only_part=0)
                emit_scores_tk(0, 0, sps, only_part=1)
                emit_scores_tk(0, 3, sps, only_part=0)
                emit_scores_tk(0, 1, sps, only_part=1)
                emit_qk_group(1, 0, 0)
                emit_scores_tk(0, 2, sps, only_part=1)
                emit_scores_tk(0, 3, sps, only_part=1)
                for g in range(1, 4):
                    emit_qk_group(0, 1, g)
                    emit_scores_tk(0, 4 * g, sps)
                    emit_scores_tk(0, 4 * g + 1, sps)
                    emit_qk_group(1, 0, g)
                    emit_scores_tk(0, 4 * g + 2, sps)
                    emit_scores_tk(0, 4 * g + 3, sps)
                # Phase C: k(hp1) + scores h1 + v
                for g in range(4):
                    emit_qk_group(1, 1, g)
                    emit_scores_tk(1, 4 * g, sps)
                    emit_v_t(4 * g)
                    emit_scores_tk(1, 4 * g + 1, sps)
                    emit_v_t(4 * g + 1)
                    emit_scores_tk(1, 4 * g + 2, sps)
                    emit_v_t(4 * g + 2)
                    emit_scores_tk(1, 4 * g + 3, sps)
                    emit_v_t(4 * g + 3)
                # Phase D: scores h2 + AV pair hp0 + transposes hp0
                for g in range(4):
                    for i in range(4):
                        emit_scores_tk(2, 4 * g + i, sps)
                        emit_av_pair(0, 4 * g + i)
                    emit_transpose_group(0, g, mmps, "qk")

            # ---- tail scope: scores h3 + AV hp1 + proj ----
            with (
                tc.tile_pool(name="sps2", bufs=2, space="PSUM") as sps2,
                tc.tile_pool(name="pps", bufs=2, space="PSUM") as pps,
                tc.tile_pool(name="outs", bufs=8) as outs,
            ):
                def emit_proj_t(t):
                    pps_t = {}
                    for n in range(2):
                        pp = pps.tile([128, 512], F32, name="pp", tag="p")
                        pps_t[n] = pp
                        nc.tensor.matmul(
                            pp[:],
                            yT_cat[:, 128 * t : 128 * t + 128],
                            wpT_sb[0][:, 512 * n : 512 * n + 512],
                            start=True,
                            stop=False,
                            skip_group_check=True,
                        )
                    for n in range(2):
                        pp = pps_t[n]
                        nc.tensor.matmul(
                            pp[:],
                            yT_cat[:, T + 128 * t : T + 128 * t + 128],
                            wpT_sb[1][:, 512 * n : 512 * n + 512],
                            start=False,
                            stop=True,
                            skip_group_check=True,
                        )
                        ot = outs.tile([128, 512], BF16, name="ot")
                        nc.vector.tensor_copy(ot[:], pp[:])
                        eng = nc.gpsimd if (t >= 14 and n == 1) else nc.sync
                        eng.dma_start(
                            out_d[
                                128 * t : 128 * t + 128,
                                512 * n : 512 * n + 512,
                            ],
                            ot[:],
                        )

                # Phase E: scores h3 (512-col psum chunks) + AV hp1 +
                # transposes hp1 + early proj pairs
                for g in range(4):
                    for i in range(4):
                        emit_scores_tk(3, 4 * g + i, sps2, part_w=1024)
                        emit_av_pair(1, 4 * g + i)
                        if g >= 1:
                            emit_proj_t(4 * (g - 1) + i)
                    emit_transpose_group(1, g, yps, "y")
                # Phase F: remaining proj
                for t in range(12, 16):
                    emit_proj_t(t)

    nc.finalize()
    return nc


def _get_nc():
    global _cached_nc
    if _cached_nc is None:
        _cached_nc = _build()
    return _cached_nc


def kernel(x, Wq, Wk, Wv, Wp, bp):
    global last_results
    x = np.asarray(x, dtype=np.float32)
    Wq = np.asarray(Wq, dtype=np.float32)
    Wk = np.asarray(Wk, dtype=np.float32)
    Wv = np.asarray(Wv, dtype=np.float32)
    Wp = np.asarray(Wp, dtype=np.float32)
    bp = np.asarray(bp, dtype=np.float32)

    E4NP = ml_dtypes.float8_e4m3
    WpT = np.ascontiguousarray(Wp.T)  # [C_in(features), C_out]
    mask01 = np.triu(np.ones((128, 128), dtype=np.float32)).astype(ml_dtypes.bfloat16)
    ident = np.eye(128, dtype=np.float32).astype(ml_dtypes.bfloat16)

    def chunked(w):
        # [C, m] -> [128, 8*m]: c-chunk c at cols [m*c : m*(c+1)]
        m = w.shape[1]
        return np.ascontiguousarray(
            w.reshape(8, 128, m).transpose(1, 0, 2).reshape(128, 8 * m)
        )

    def hilo(a):
        hi = a.astype(E4NP)
        lo = (a - hi.astype(np.float32)).astype(E4NP)
        return hi, lo

    xT_by_batch = [np.ascontiguousarray(x[b].T) for b in range(B)]
    xhl_by_batch = [hilo(chunked(xT_by_batch[b])) for b in range(B)]

    in_maps = []
    for core in range(N_CORES):
        b, g = core // 4, core % 4
        h0 = HPC * g
        def wq_pair(W):
            res = []
            for hp in range(2):
                cat = np.concatenate(
                    [W[h0 + 2 * hp], W[h0 + 2 * hp + 1]], axis=1
                ) * WS
                h_, l_ = hilo(chunked(cat))
                res.append(np.concatenate([h_, l_], axis=1))
            return np.stack(res)
        wq_p = wq_pair(Wq)
        wk_p = wq_pair(Wk)
        wv_cat = np.concatenate([Wv[h0 + j] for j in range(HPC)], axis=1) * WS
        wvh, wvl = hilo(chunked(wv_cat))
        wv_p = np.concatenate([wvh, wvl], axis=1)
        wpT_p = np.ascontiguousarray(
            WpT[256 * g : 256 * (g + 1)].reshape(2, 128, C)
        ).astype(ml_dtypes.bfloat16)
        xh, xl = xhl_by_batch[b]
        in_maps.append(
            {
                "xh": xh, "xl": xl,
                "wq": wq_p, "wk": wk_p, "wv": wv_p,
                "wpT": wpT_p,
                "mask": mask01,
                "ident": ident,
            }
        )

    nc = _get_nc()
    kwargs = {}
    if os.environ.get("KERNEL_TRACE", "0") == "1":
        kwargs = dict(trace=True, trace_cores=list(range(N_CORES)),
                      stitch_traces=True)
    try:
        res = run_bass_kernel_spmd(
            nc, in_maps, core_ids=list(range(N_CORES)), **kwargs
        )
    except ModuleNotFoundError:
        res = run_bass_kernel_spmd(nc, in_maps, core_ids=list(range(N_CORES)))
    last_results = res

    out = np.zeros((B, T, C), dtype=np.float32)
    for core in range(N_CORES):
        b = core // 4
        out[b] += res.results[core]["out"].astype(np.float32)
    out += bp[None, None, :]
    return out


# revision 27
# speedup vs baseline: 1.0802x; 1.0142x over previous
"""Multi-head causal attention (B=2, T=2048, C=1024, H=16, S=64) on 8 TRN2 cores.

Sharding: core i handles batch b = i//4 and head group g = i%4 (4 heads each).
Each core computes a partial output projection (its heads' contribution to the
full [T, C] output); the host sums the 4 partials per batch and adds the bias.

V2 dataflow (cost model: matmul time = out_free_cols x cycles(moving dtype),
fp8e4 DoubleRow = 0.5 cycles/col):
  - QKV projections: error-compensated fp8 (x = xh+xl, W*32 = wh+wl; terms
    xh*wh + xh*wl + xl*wh via DoubleRow chunk pairs; xl*wl dropped).
    12 DR matmuls replace 8 bf16 matmuls per tile: 25% fewer PE cycles.
  - Scores: q,k quantized to fp8; off-diagonal tiles via zero-padded
    DoubleRow (stationary = (k_tile, zeros), moving = q twice, stride-0) at
    0.5 cycles/col. Diagonal tiles in bf16 for accuracy (softmax spike).
  - p = exp(s * 0.125/1024) on ACT (weights carry x32 scale per side).
  - AV reoriented: stationary = p tile [u,tq], moving = v|1 [u,65]; out
    y[tq, 65] accumulated over u tiles in PSUM (col 64 = denominator).
  - Normalize y by 1/d per (pair, tq-tile) on DVE (stride-0 broadcast mul),
    transpose y -> yT via PE matmul against identity.
  - Output projection bf16 (stationary yT, moving WpT), accumulate head
    pairs in PSUM.
"""

import os
import math
import numpy as np
import ml_dtypes

import concourse.bacc as bacc
import concourse.mybir as mybir
import concourse.tile as tile
from concourse.bass_utils import run_bass_kernel_spmd

F32 = mybir.dt.float32
BF16 = mybir.dt.bfloat16
E4 = mybir.dt.float8e4

B, T, C, H, S = 2, 2048, 1024, 16, 64
HPC = 4          # heads per core
N_CORES = 8
NC_T = T // 128  # 16 t-tiles of 128
WS = 32.0        # weight scale folded into fp8 weights

# p storage offsets: tile tk spans tq in [128*tk, 2048)
SPAN = [T - 128 * tk for tk in range(NC_T)]
OFF = [0] * NC_T
for _tk in range(1, NC_T):
    OFF[_tk] = OFF[_tk - 1] + SPAN[_tk - 1]
ATT_W = OFF[-1] + SPAN[-1]  # 17408

_cached_nc = None
last_results = None  # BassKernelResults of the most recent run (for test harness)


def _build():
    nc = bacc.Bacc("TRN2", target_bir_lowering=False)

    # fp8 hi/lo inputs, chunk-major so each DMA is contiguous per partition.
    xh_d = nc.dram_tensor("xh", [128, 8 * 2048], E4, kind="ExternalInput")
    xl_d = nc.dram_tensor("xl", [128, 8 * 2048], E4, kind="ExternalInput")
    wq_d = nc.dram_tensor("wq", [2, 128, 2 * 8 * 128], E4, kind="ExternalInput")
    wk_d = nc.dram_tensor("wk", [2, 128, 2 * 8 * 128], E4, kind="ExternalInput")
    wv_d = nc.dram_tensor("wv", [128, 2 * 8 * 256], E4, kind="ExternalInput")
    wpT_d = nc.dram_tensor("wpT", [2, 128, C], BF16, kind="ExternalInput")
    mask_d = nc.dram_tensor("mask", [128, 128], BF16, kind="ExternalInput")
    ident_d = nc.dram_tensor("ident", [128, 128], BF16, kind="ExternalInput")
    out_d = nc.dram_tensor("out", [T, C], BF16, kind="ExternalOutput")

    with tile.TileContext(nc) as tc:
        with (
            tc.tile_pool(name="const", bufs=1) as constp,
            tc.tile_pool(name="qk8", bufs=1) as qk8p,
            tc.tile_pool(name="qk16", bufs=1) as qk16p,
            tc.tile_pool(name="vsb", bufs=1) as vp,
            tc.tile_pool(name="ysb", bufs=1) as ysbp,
            tc.tile_pool(name="yT", bufs=1) as ytcp,
            tc.tile_pool(name="attp", bufs=1) as attp,
            tc.tile_pool(name="sm", bufs=2) as smp,
            tc.tile_pool(name="ypsum", bufs=2, space="PSUM") as yps,
        ):
            # persistent tiles
            mask_sb = constp.tile([128, 128], BF16, name="mask_sb")
            nc.gpsimd.dma_start(mask_sb[:], mask_d[:])
            ident_sb = constp.tile([128, 128], BF16, name="ident_sb")
            nc.gpsimd.dma_start(ident_sb[:], ident_d[:])

            # fp8 q/k: q gets 128 zero-pad cols (moving side of the
            # zero-padded DoubleRow score matmuls; q writes complete early
            # so the wide moving AP causes no late false deps)
            q8 = [qk8p.tile([128, T + 128], E4, name=f"q8_{hp}") for hp in range(2)]
            k8 = [qk8p.tile([128, T], E4, name=f"k8_{hp}") for hp in range(2)]
            for hp in range(2):
                nc.vector.memset(q8[hp][:, T : T + 128], 0.0)
            # bf16 q/k for the diagonal score tiles
            q16 = [qk16p.tile([128, T], BF16, name=f"q16_{hp}") for hp in range(2)]
            k16 = [qk16p.tile([128, T], BF16, name=f"k16_{hp}") for hp in range(2)]

            # v tiles: [128, 4*65] bf16; head h in cols 65h..65h+63, col 65h+64 = 1
            v_sb = [vp.tile([128, 4 * 65], BF16, name=f"v{t}") for t in range(NC_T)]
            for t in range(NC_T):
                ones_ap = v_sb[t].rearrange("p (h c) -> p h c", h=4)[:, :, 64]
                nc.vector.memset(ones_ap, 1.0)

            # normalized y staging [tq, (h_even|h_odd)] per pair, per tq tile
            y_sb = [ysbp.tile([128, NC_T * 128], BF16, name=f"ysb{hp}")
                    for hp in range(2)]
            # concatenated yT for proj: dim1 = hp
            yT_cat = ytcp.tile([128, 2 * T], BF16, name="yT_cat")

            # p buffers (3-deep head pipeline: exp(h+2) must not wait on
            # AV(h) finishing)
            att_buf = [attp.tile([128, ATT_W], BF16, name=f"attb{i}")
                       for i in range(3)]
            BUF_OF = [0, 1, 2, 0]  # head -> p buffer

            wpT_sb = [constp.tile([128, C], BF16, name=f"wpT{hp}")
                      for hp in range(2)]

            # Schraudolph fast-exp constants (bf16 bit trick):
            # bits_i16 = A*z + B with z = s_psum * 0.125/WS^2
            SCH_A = 184.66496 * 0.125 / (WS * WS)
            SCH_B = 16252.0
            exp_ctr = [0]

            def emit_exp(dst, src):
                exp_ctr[0] += 1
                if exp_ctr[0] % 8 == 0:
                    # DVE fast-exp: affine into int16, bitcast to bf16
                    nc.vector.tensor_scalar(
                        dst.bitcast(mybir.dt.int16),
                        src,
                        SCH_A,
                        SCH_B,
                        mybir.AluOpType.mult,
                        mybir.AluOpType.add,
                    )
                else:
                    nc.scalar.activation(
                        dst, src,
                        mybir.ActivationFunctionType.Exp,
                        scale=0.125 / (WS * WS),
                    )

            def emit_scores_tk(h, tk, sps_pool, part_w=1024, only_part=None):
                """Scores for head h, k-tile tk: diag 128 cols in bf16 +
                off-diag in zero-padded fp8 DR chunks; exp into att_buf."""
                hp, half = h // 2, h % 2
                r0 = 64 * half
                ab = att_buf[BUF_OF[h]]
                span = SPAN[tk]
                kt16 = k16[hp][r0 : r0 + 64, 128 * tk : 128 * tk + 128]
                # stationary fp8: (k tile, k tile) via stride-0 broadcast;
                # the moving q side supplies (q cols, zeros) so the second
                # k contribution is k.T @ 0 = 0.
                kt8 = (
                    k8[hp][r0 : r0 + 64, 128 * tk : 128 * tk + 128]
                    .unsqueeze(1)
                    .broadcast_to([64, 2, 128])
                )
                for part in range(math.ceil(span / part_w)):
                    if only_part is not None and part != only_part:
                        continue
                    pspan = min(part_w, span - part_w * part)
                    pt = sps_pool.tile([128, part_w], F32, name="sps_t", tag="s")
                    c0 = 0
                    if part == 0:
                        # diagonal block in bf16 (K=64)
                        nc.tensor.matmul(
                            pt[:, 0:128],
                            kt16,
                            q16[hp][r0 : r0 + 64, 128 * tk : 128 * tk + 128],
                            start=True,
                            stop=True,
                        )
                        c0 = 128
                    while c0 < pspan:
                        n = min(128, pspan - c0)
                        tq0 = 128 * tk + part_w * part + c0
                        # moving: (q cols tq0.., zero pad at col T), built by
                        # 128-col rechunking + stride slicing
                        nch = (T + 128 - tq0) // 128
                        qmov = (
                            q8[hp][r0 : r0 + 64, tq0 : T + 128]
                            .rearrange("p (x m) -> p x m", x=nch)[:, 0 :: max(nch - 1, 1), :]
                        )
                        if nch == 1:
                            qmov = qmov.broadcast_to([64, 2, n])
                        nc.tensor.matmul(
                            pt[:, c0 : c0 + n],
                            kt8,
                            qmov[:, :, 0:n] if n != 128 else qmov,
                            start=True,
                            stop=True,
                            perf_mode=mybir.MatmulPerfMode.DoubleRow,
                        )
                        c0 += n
                    dst = ab[
                        :, OFF[tk] + part_w * part : OFF[tk] + part_w * part + pspan
                    ]
                    emit_exp(dst, pt[:, 0:pspan])
                # mask the diagonal block (first 128 cols of this tk tile)
                diag = ab[:, OFF[tk] : OFF[tk] + 128]
                nc.gpsimd.tensor_mul(diag, diag, mask_sb[:])

            def emit_av_pair(hp, j):
                """y[tq, 65] for both heads of pair hp, tq-tile j, then
                normalize into y_sb[hp] block j."""
                yp = yps.tile([128, 512], F32, name="yps_t", tag="y")
                for half in range(2):
                    h = 2 * hp + half
                    ab = att_buf[BUF_OF[h]]
                    for tk in range(j + 1):
                        ptile = ab[:, OFF[tk] + 128 * (j - tk) : OFF[tk] + 128 * (j - tk) + 128]
                        nc.tensor.matmul(
                            yp[:, 65 * half : 65 * half + 65],
                            ptile,
                            v_sb[tk][:, 65 * h : 65 * h + 65],
                            start=(tk == 0),
                            stop=(tk == j),
                            skip_group_check=True,
                        )
                # normalize: y into y_sb[hp] block j, bf16
                ypv = yp[:, 0:130].rearrange("p (h c) -> p h c", h=2)
                rec = smp.tile([128, 2], F32, name="rec")
                nc.vector.reciprocal(rec[:], ypv[:, :, 64])
                dst = (
                    y_sb[hp][:, 128 * j : 128 * j + 128]
                    .rearrange("p (h c) -> p h c", h=2)
                )
                nc.vector.tensor_mul(
                    dst,
                    ypv[:, :, 0:64],
                    rec[:].unsqueeze(2).broadcast_to([128, 2, 64]),
                )

            def emit_transpose_group(hp, jg, tp_pool, tp_tag):
                """Transpose y_sb[hp] tiles 4jg..4jg+3 into yT_cat via PE
                matmul with identity; copy PSUM->SBUF bf16 on ACT."""
                tp = tp_pool.tile([128, 512], F32, name="ytp_t", tag=tp_tag)
                for jj in range(4):
                    j = 4 * jg + jj
                    nc.tensor.matmul(
                        tp[:, 128 * jj : 128 * jj + 128],
                        y_sb[hp][:, 128 * j : 128 * j + 128],
                        ident_sb[:],
                        start=True,
                        stop=True,
                        skip_group_check=True,
                    )
                nc.vector.tensor_copy(
                    yT_cat[:, T * hp + 512 * jg : T * hp + 512 * jg + 512], tp[:]
                )

            # ---- scores/QKV scope ----
            with (
                tc.tile_pool(name="sps", bufs=2, space="PSUM") as sps,
                tc.tile_pool(name="xw", bufs=1) as xw,
                tc.tile_pool(name="mmps", bufs=2, space="PSUM") as mmps,
            ):
                xh_sb = xw.tile([128, 8 * 2048], E4, name="xh")
                xl_sb = xw.tile([128, 8 * 2048], E4, name="xl")
                wq_sb = [xw.tile([128, 2 * 8 * 128], E4, name=f"wq{hp}")
                         for hp in range(2)]
                wk_sb = [xw.tile([128, 2 * 8 * 128], E4, name=f"wk{hp}")
                         for hp in range(2)]
                wv_sb = xw.tile([128, 2 * 8 * 256], E4, name="wv")

                # x + weights on SP HWDGE and SWDGE only: the ACT queue must
                # stay clear, DMA dispatches there would serialize with exp
                # on the ACT sequencer.
                xh_r = xh_sb.rearrange("p (c n) -> p c n", c=8)
                xl_r = xl_sb.rearrange("p (c n) -> p c n", c=8)
                xhd_r = xh_d[:].rearrange("p (c n) -> p c n", c=8)
                xld_r = xl_d[:].rearrange("p (c n) -> p c n", c=8)
                # all of x on SP in consumption order, few big transfers
                # (each DMACopy pays ~625ns HWDGE generation serially)
                nc.sync.dma_start(wq_sb[0][:], wq_d[0])
                nc.sync.dma_start(xh_sb[:, 0 : 4 * 2048], xh_d[:, 0 : 4 * 2048])
                nc.sync.dma_start(xh_sb[:, 4 * 2048 :], xh_d[:, 4 * 2048 :])
                nc.sync.dma_start(wk_sb[0][:], wk_d[0])
                nc.sync.dma_start(xl_sb[:, 0 : 4 * 2048], xl_d[:, 0 : 4 * 2048])
                nc.sync.dma_start(xl_sb[:, 4 * 2048 :], xl_d[:, 4 * 2048 :])
                nc.sync.dma_start(wq_sb[1][:], wq_d[1])
                nc.sync.dma_start(wk_sb[1][:], wk_d[1])
                nc.sync.dma_start(wv_sb[:], wv_d[:])

                def emit_qk_group(hp, kind, tq):
                    """q or k for head pair hp, 512 t-cols starting 512*tq.
                    Compensated fp8: xh*wh + xh*wl + xl*wh, DR chunk pairs.
                    One DVE fp8 copy + one Pool bf16 copy per group."""
                    w_sb = wq_sb if kind == 0 else wk_sb
                    d8 = q8[hp] if kind == 0 else k8[hp]
                    d16 = q16[hp] if kind == 0 else k16[hp]
                    pt = mmps.tile([128, 512], F32, name="qkps", tag="qk")
                    for half in range(2):
                        n0 = 512 * tq + 256 * half
                        first = True
                        for xs, wl_ in ((xh_r, 0), (xh_r, 1), (xl_r, 0)):
                            wr = w_sb[hp][:, 1024 * wl_ : 1024 * wl_ + 1024].rearrange(
                                "p (c m) -> p c m", c=8)
                            for cp in range(4):
                                nc.tensor.matmul(
                                    pt[:, 256 * half : 256 * half + 256],
                                    wr[:, 2 * cp : 2 * cp + 2, :],
                                    xs[:, 2 * cp : 2 * cp + 2, n0 : n0 + 256],
                                    start=first,
                                    stop=(wl_ == 0 and xs is xl_r and cp == 3),
                                    perf_mode=mybir.MatmulPerfMode.DoubleRow,
                                )
                                first = False
                    n0 = 512 * tq
                    # Pool cannot read PSUM: bf16 from PSUM on DVE, then
                    # fp8 from the bf16 copy on Pool (SBUF->SBUF)
                    nc.vector.tensor_copy(d16[:, n0 : n0 + 512], pt[:])
                    nc.gpsimd.tensor_copy(
                        d8[:, n0 : n0 + 512], d16[:, n0 : n0 + 512]
                    )

                def emit_v_t(t):
                    pv = mmps.tile([128, 512], F32, name="vps", tag="qk")[:, 0:256]
                    first = True
                    for xs, wl_ in ((xh_r, 0), (xh_r, 1), (xl_r, 0)):
                        wr = wv_sb[:, 2048 * wl_ : 2048 * wl_ + 2048].rearrange(
                            "p (c m) -> p c m", c=8)
                        for cp in range(4):
                            nc.tensor.matmul(
                                pv[:],
                                xs[:, 2 * cp : 2 * cp + 2, 128 * t : 128 * t + 128],
                                wr[:, 2 * cp : 2 * cp + 2, :],
                                start=first,
                                stop=(wl_ == 0 and xs is xl_r and cp == 3),
                                perf_mode=mybir.MatmulPerfMode.DoubleRow,
                            )
                            first = False
                    # v = pv / WS, bf16
                    nc.vector.tensor_scalar_mul(
                        v_sb[t].rearrange("p (h c) -> p h c", h=4)[:, :, 0:64],
                        pv[:].rearrange("p (h c) -> p h c", h=4),
                        1.0 / WS,
                    )

                # PE warm-up on the mask tile while input DMAs land.
                warm = sps.tile([128, 1024], F32, name="warm", tag="s")
                for i in range(16):
                    nc.tensor.matmul(
                        warm[:, 0:128], mask_sb[:], mask_sb[:],
                        start=True, stop=True,
                    )
                # Phase A: q(hp0) groups 0,1 then k(hp0) group 0, so the
                # first score parts (tq < 1024) can start before all of q
                # is projected.
                emit_qk_group(0, 0, 0)
                emit_qk_group(0, 0, 1)
                emit_qk_group(0, 1, 0)
                for hp in range(2):
                    nc.gpsimd.dma_start(wpT_sb[hp][:], wpT_d[hp])
                # Phase B head: interleave remaining q groups with the
                # early score parts of tiles 0..3
                emit_scores_tk(0, 0, sps, only_part=0)
                emit_qk_group(0, 0, 2)
                emit_scores_tk(0, 1, sps, only_part=0)
                emit_qk_group(0, 0, 3)
                emit_scores_tk(0, 2, sps, only_part=0)
                emit_scores_tk(0, 0, sps, only_part=1)
                emit_scores_tk(0, 3, sps, only_part=0)
                emit_scores_tk(0, 1, sps, only_part=1)
                emit_qk_group(1, 0, 0)
                emit_scores_tk(0, 2, sps, only_part=1)
                emit_scores_tk(0, 3, sps, only_part=1)
                for g in range(1, 4):
                    emit_qk_group(0, 1, g)
                    emit_scores_tk(0, 4 * g, sps)
                    emit_scores_tk(0, 4 * g + 1, sps)
                    emit_qk_group(1, 0, g)
                    emit_scores_tk(0, 4 * g + 2, sps)
                    emit_scores_tk(0, 4 * g + 3, sps)
                # Phase C: k(hp1) + scores h1 + v
                for g in range(4):
                    emit_qk_group(1, 1, g)
                    emit_scores_tk(1, 4 * g, sps)
                    emit_v_t(4 * g)
                    emit_scores_tk(1, 4 * g + 1, sps)
                    emit_v_t(4 * g + 1)
                    emit_scores_tk(1, 4 * g + 2, sps)
                    emit_v_t(4 * g + 2)
                    emit_scores_tk(1, 4 * g + 3, sps)
                    emit_v_t(4 * g + 3)
                # Phase D: scores h2 + AV pair hp0 + transposes hp0
                for g in range(4):
                    for i in range(4):
                        emit_scores_tk(2, 4 * g + i, sps)
                        emit_av_pair(0, 4 * g + i)
                    emit_transpose_group(0, g, mmps, "qk")

            # ---- tail scope: scores h3 + AV hp1 + proj ----
            with (
                tc.tile_pool(name="sps2", bufs=2, space="PSUM") as sps2,
                tc.tile_pool(name="pps", bufs=2, space="PSUM") as pps,
                tc.tile_pool(name="outs", bufs=8) as outs,
            ):
                def emit_proj_t(t):
                    pps_t = {}
                    for n in range(2):
                        pp = pps.tile([128, 512], F32, name="pp", tag="p")
                        pps_t[n] = pp
                        nc.tensor.matmul(
                            pp[:],
                            yT_cat[:, 128 * t : 128 * t + 128],
                            wpT_sb[0][:, 512 * n : 512 * n + 512],
                            start=True,
                            stop=False,
                            skip_group_check=True,
                        )
                    for n in range(2):
                        pp = pps_t[n]
                        nc.tensor.matmul(
                            pp[:],
                            yT_cat[:, T + 128 * t : T + 128 * t + 128],
                            wpT_sb[1][:, 512 * n : 512 * n + 512],
                            start=False,
                            stop=True,
                            skip_group_check=True,
                        )
                        ot = outs.tile([128, 512], BF16, name="ot")
                        nc.vector.tensor_copy(ot[:], pp[:])
                        eng = nc.gpsimd if (t >= 14 and n == 1) else nc.sync
                        eng.dma_start(
                            out_d[
                                128 * t : 128 * t + 128,
                                512 * n : 512 * n + 512,
                            ],
                            ot[:],
                        )

                # Phase E: scores h3 (512-col psum chunks) + AV hp1 +
                # transposes hp1 + early proj pairs
                for g in range(4):
                    for i in range(4):
                        j = 4 * g + i
                        emit_scores_tk(3, j, sps2, part_w=1024)
                        emit_av_pair(1, j)
                        # per-tile transpose: shortens the post-exp tail
                        tp = yps.tile([128, 512], F32, name="ytp_t", tag="y")
                        nc.tensor.matmul(
                            tp[:, 0:128],
                            y_sb[1][:, 128 * j : 128 * j + 128],
                            ident_sb[:],
                            start=True,
                            stop=True,
                            skip_group_check=True,
                        )
                        nc.vector.tensor_copy(
                            yT_cat[:, T + 128 * j : T + 128 * j + 128],
                            tp[:, 0:128],
                        )
                        if g >= 1:
                            emit_proj_t(4 * (g - 1) + i)
                # Phase F: remaining proj
                for t in range(12, 16):
                    emit_proj_t(t)

    nc.finalize()
    return nc


def _get_nc():
    global _cached_nc
    if _cached_nc is None:
        _cached_nc = _build()
    return _cached_nc


def kernel(x, Wq, Wk, Wv, Wp, bp):
    global last_results
    x = np.asarray(x, dtype=np.float32)
    Wq = np.asarray(Wq, dtype=np.float32)
    Wk = np.asarray(Wk, dtype=np.float32)
    Wv = np.asarray(Wv, dtype=np.float32)
    Wp = np.asarray(Wp, dtype=np.float32)
    bp = np.asarray(bp, dtype=np.float32)

    E4NP = ml_dtypes.float8_e4m3
    WpT = np.ascontiguousarray(Wp.T)  # [C_in(features), C_out]
    mask01 = np.triu(np.ones((128, 128), dtype=np.float32)).astype(ml_dtypes.bfloat16)
    ident = np.eye(128, dtype=np.float32).astype(ml_dtypes.bfloat16)

    def chunked(w):
        # [C, m] -> [128, 8*m]: c-chunk c at cols [m*c : m*(c+1)]
        m = w.shape[1]
        return np.ascontiguousarray(
            w.reshape(8, 128, m).transpose(1, 0, 2).reshape(128, 8 * m)
        )

    def hilo(a):
        hi = a.astype(E4NP)
        lo = (a - hi.astype(np.float32)).astype(E4NP)
        return hi, lo

    xT_by_batch = [np.ascontiguousarray(x[b].T) for b in range(B)]
    xhl_by_batch = [hilo(chunked(xT_by_batch[b])) for b in range(B)]

    in_maps = []
    for core in range(N_CORES):
        b, g = core // 4, core % 4
        h0 = HPC * g
        def wq_pair(W):
            res = []
            for hp in range(2):
                cat = np.concatenate(
                    [W[h0 + 2 * hp], W[h0 + 2 * hp + 1]], axis=1
                ) * WS
                h_, l_ = hilo(chunked(cat))
                res.append(np.concatenate([h_, l_], axis=1))
            return np.stack(res)
        wq_p = wq_pair(Wq)
        wk_p = wq_pair(Wk)
        wv_cat = np.concatenate([Wv[h0 + j] for j in range(HPC)], axis=1) * WS
        wvh, wvl = hilo(chunked(wv_cat))
        wv_p = np.concatenate([wvh, wvl], axis=1)
        wpT_p = np.ascontiguousarray(
            WpT[256 * g : 256 * (g + 1)].reshape(2, 128, C)
        ).astype(ml_dtypes.bfloat16)
        xh, xl = xhl_by_batch[b]
        in_maps.append(
            {
                "xh": xh, "xl": xl,
                "wq": wq_p, "wk": wk_p, "wv": wv_p,
                "wpT": wpT_p,
                "mask": mask01,
                "ident": ident,
            }
        )

    nc = _get_nc()
    kwargs = {}
    if os.environ.get("KERNEL_TRACE", "0") == "1":
        kwargs = dict(trace=True, trace_cores=list(range(N_CORES)),
                      stitch_traces=True)
    try:
        res = run_bass_kernel_spmd(
            nc, in_maps, core_ids=list(range(N_CORES)), **kwargs
        )
    except ModuleNotFoundError:
        res = run_bass_kernel_spmd(nc, in_maps, core_ids=list(range(N_CORES)))
    last_results = res

    out = np.zeros((B, T, C), dtype=np.float32)
    for core in range(N_CORES):
        b = core // 4
        out[b] += res.results[core]["out"].astype(np.float32)
    out += bp[None, None, :]
    return out


# revision 30
# speedup vs baseline: 1.0980x; 1.0165x over previous
"""Multi-head causal attention (B=2, T=2048, C=1024, H=16, S=64) on 8 TRN2 cores.

Sharding: core i handles batch b = i//4 and head group g = i%4 (4 heads each).
Each core computes a partial output projection (its heads' contribution to the
full [T, C] output); the host sums the 4 partials per batch and adds the bias.

V2 dataflow (cost model: matmul time = out_free_cols x cycles(moving dtype),
fp8e4 DoubleRow = 0.5 cycles/col):
  - QKV projections: error-compensated fp8 (x = xh+xl, W*32 = wh+wl; terms
    xh*wh + xh*wl + xl*wh via DoubleRow chunk pairs; xl*wl dropped).
    12 DR matmuls replace 8 bf16 matmuls per tile: 25% fewer PE cycles.
  - Scores: q,k quantized to fp8; off-diagonal tiles via zero-padded
    DoubleRow (stationary = (k_tile, zeros), moving = q twice, stride-0) at
    0.5 cycles/col. Diagonal tiles in bf16 for accuracy (softmax spike).
  - p = exp(s * 0.125/1024) on ACT (weights carry x32 scale per side).
  - AV reoriented: stationary = p tile [u,tq], moving = v|1 [u,65]; out
    y[tq, 65] accumulated over u tiles in PSUM (col 64 = denominator).
  - Normalize y by 1/d per (pair, tq-tile) on DVE (stride-0 broadcast mul),
    transpose y -> yT via PE matmul against identity.
  - Output projection bf16 (stationary yT, moving WpT), accumulate head
    pairs in PSUM.
"""

import os
import math
import numpy as np
import ml_dtypes

import concourse.bacc as bacc
import concourse.mybir as mybir
import concourse.tile as tile
from concourse.bass_utils import run_bass_kernel_spmd

F32 = mybir.dt.float32
BF16 = mybir.dt.bfloat16
E4 = mybir.dt.float8e4

B, T, C, H, S = 2, 2048, 1024, 16, 64
HPC = 4          # heads per core
N_CORES = 8
NC_T = T // 128  # 16 t-tiles of 128
WS = 32.0        # weight scale folded into fp8 weights

# p storage offsets: tile tk spans tq in [128*tk, 2048)
SPAN = [T - 128 * tk for tk in range(NC_T)]
OFF = [0] * NC_T
for _tk in range(1, NC_T):
    OFF[_tk] = OFF[_tk - 1] + SPAN[_tk - 1]
ATT_W = OFF[-1] + SPAN[-1]  # 17408

_cached_nc = None
last_results = None  # BassKernelResults of the most recent run (for test harness)


def _build():
    nc = bacc.Bacc("TRN2", target_bir_lowering=False)

    # fp8 hi/lo inputs, chunk-major so each DMA is contiguous per partition.
    xh_d = nc.dram_tensor("xh", [128, 8 * 2048], E4, kind="ExternalInput")
    xl_d = nc.dram_tensor("xl", [128, 8 * 2048], E4, kind="ExternalInput")
    wq_d = nc.dram_tensor("wq", [2, 128, 2 * 8 * 128], E4, kind="ExternalInput")
    wk_d = nc.dram_tensor("wk", [2, 128, 2 * 8 * 128], E4, kind="ExternalInput")
    wv_d = nc.dram_tensor("wv", [128, 2 * 8 * 256], E4, kind="ExternalInput")
    wpT_d = nc.dram_tensor("wpT", [2, 128, C], BF16, kind="ExternalInput")
    mask_d = nc.dram_tensor("mask", [128, 128], BF16, kind="ExternalInput")
    ident_d = nc.dram_tensor("ident", [128, 128], BF16, kind="ExternalInput")
    out_d = nc.dram_tensor("out", [T, C], BF16, kind="ExternalOutput")

    with tile.TileContext(nc) as tc:
        with (
            tc.tile_pool(name="const", bufs=1) as constp,
            tc.tile_pool(name="qk8", bufs=1) as qk8p,
            tc.tile_pool(name="qk16", bufs=1) as qk16p,
            tc.tile_pool(name="vsb", bufs=1) as vp,
            tc.tile_pool(name="ysb", bufs=1) as ysbp,
            tc.tile_pool(name="yT", bufs=1) as ytcp,
            tc.tile_pool(name="attp", bufs=1) as attp,
            tc.tile_pool(name="sm", bufs=2) as smp,
            tc.tile_pool(name="ypsum", bufs=2, space="PSUM") as yps,
        ):
            # persistent tiles
            mask_sb = constp.tile([128, 128], BF16, name="mask_sb")
            nc.gpsimd.dma_start(mask_sb[:], mask_d[:])
            ident_sb = constp.tile([128, 128], BF16, name="ident_sb")
            nc.gpsimd.dma_start(ident_sb[:], ident_d[:])

            # fp8 q/k: q gets 128 zero-pad cols (moving side of the
            # zero-padded DoubleRow score matmuls; q writes complete early
            # so the wide moving AP causes no late false deps)
            q8 = [qk8p.tile([128, T + 128], E4, name=f"q8_{hp}") for hp in range(2)]
            k8 = [qk8p.tile([128, T], E4, name=f"k8_{hp}") for hp in range(2)]
            for hp in range(2):
                nc.vector.memset(q8[hp][:, T : T + 128], 0.0)
            # bf16 q/k for the diagonal score tiles
            q16 = [qk16p.tile([128, T], BF16, name=f"q16_{hp}") for hp in range(2)]
            k16 = [qk16p.tile([128, T], BF16, name=f"k16_{hp}") for hp in range(2)]

            # v tiles: [128, 4*65] bf16; head h in cols 65h..65h+63, col 65h+64 = 1
            v_sb = [vp.tile([128, 4 * 65], BF16, name=f"v{t}") for t in range(NC_T)]
            for t in range(NC_T):
                ones_ap = v_sb[t].rearrange("p (h c) -> p h c", h=4)[:, :, 64]
                nc.vector.memset(ones_ap, 1.0)

            # normalized y staging [tq, (h_even|h_odd)] per pair, per tq tile
            y_sb = [ysbp.tile([128, NC_T * 128], BF16, name=f"ysb{hp}")
                    for hp in range(2)]
            # concatenated yT for proj: dim1 = hp
            yT_cat = ytcp.tile([128, 2 * T], BF16, name="yT_cat")

            # p buffers (3-deep head pipeline: exp(h+2) must not wait on
            # AV(h) finishing)
            att_buf = [attp.tile([128, ATT_W], BF16, name=f"attb{i}")
                       for i in range(3)]
            BUF_OF = [0, 1, 2, 0]  # head -> p buffer

            wpT_sb = [constp.tile([128, C], BF16, name=f"wpT{hp}")
                      for hp in range(2)]

            # Schraudolph fast-exp constants (bf16 bit trick):
            # bits_i16 = A*z + B with z = s_psum * 0.125/WS^2
            SCH_A = 184.66496 * 0.125 / (WS * WS)
            SCH_B = 16252.0
            exp_ctr = [0]

            def emit_exp(dst, src):
                exp_ctr[0] += 1
                if exp_ctr[0] % 6 == 0:
                    # DVE fast-exp: affine into int16, bitcast to bf16
                    nc.vector.tensor_scalar(
                        dst.bitcast(mybir.dt.int16),
                        src,
                        SCH_A,
                        SCH_B,
                        mybir.AluOpType.mult,
                        mybir.AluOpType.add,
                    )
                else:
                    nc.scalar.activation(
                        dst, src,
                        mybir.ActivationFunctionType.Exp,
                        scale=0.125 / (WS * WS),
                    )

            def emit_scores_tk(h, tk, sps_pool, part_w=1024, only_part=None):
                """Scores for head h, k-tile tk: diag 128 cols in bf16 +
                off-diag in zero-padded fp8 DR chunks; exp into att_buf."""
                hp, half = h // 2, h % 2
                r0 = 64 * half
                ab = att_buf[BUF_OF[h]]
                span = SPAN[tk]
                kt16 = k16[hp][r0 : r0 + 64, 128 * tk : 128 * tk + 128]
                # stationary fp8: (k tile, k tile) via stride-0 broadcast;
                # the moving q side supplies (q cols, zeros) so the second
                # k contribution is k.T @ 0 = 0.
                kt8 = (
                    k8[hp][r0 : r0 + 64, 128 * tk : 128 * tk + 128]
                    .unsqueeze(1)
                    .broadcast_to([64, 2, 128])
                )
                for part in range(math.ceil(span / part_w)):
                    if only_part is not None and part != only_part:
                        continue
                    pspan = min(part_w, span - part_w * part)
                    pt = sps_pool.tile([128, part_w], F32, name="sps_t", tag="s")
                    c0 = 0
                    if part == 0:
                        # diagonal block in bf16 (K=64)
                        nc.tensor.matmul(
                            pt[:, 0:128],
                            kt16,
                            q16[hp][r0 : r0 + 64, 128 * tk : 128 * tk + 128],
                            start=True,
                            stop=True,
                        )
                        c0 = 128
                    while c0 < pspan:
                        n = min(128, pspan - c0)
                        tq0 = 128 * tk + part_w * part + c0
                        # moving: (q cols tq0.., zero pad at col T), built by
                        # 128-col rechunking + stride slicing
                        nch = (T + 128 - tq0) // 128
                        qmov = (
                            q8[hp][r0 : r0 + 64, tq0 : T + 128]
                            .rearrange("p (x m) -> p x m", x=nch)[:, 0 :: max(nch - 1, 1), :]
                        )
                        if nch == 1:
                            qmov = qmov.broadcast_to([64, 2, n])
                        nc.tensor.matmul(
                            pt[:, c0 : c0 + n],
                            kt8,
                            qmov[:, :, 0:n] if n != 128 else qmov,
                            start=True,
                            stop=True,
                            perf_mode=mybir.MatmulPerfMode.DoubleRow,
                        )
                        c0 += n
                    dst = ab[
                        :, OFF[tk] + part_w * part : OFF[tk] + part_w * part + pspan
                    ]
                    emit_exp(dst, pt[:, 0:pspan])
                # mask the diagonal block (first 128 cols of this tk tile)
                diag = ab[:, OFF[tk] : OFF[tk] + 128]
                nc.gpsimd.tensor_mul(diag, diag, mask_sb[:])

            def emit_av_pair(hp, j):
                """y[tq, 65] for both heads of pair hp, tq-tile j, then
                normalize into y_sb[hp] block j."""
                yp = yps.tile([128, 512], F32, name="yps_t", tag="y")
                for half in range(2):
                    h = 2 * hp + half
                    ab = att_buf[BUF_OF[h]]
                    for tk in range(j + 1):
                        ptile = ab[:, OFF[tk] + 128 * (j - tk) : OFF[tk] + 128 * (j - tk) + 128]
                        nc.tensor.matmul(
                            yp[:, 65 * half : 65 * half + 65],
                            ptile,
                            v_sb[tk][:, 65 * h : 65 * h + 65],
                            start=(tk == 0),
                            stop=(tk == j),
                            skip_group_check=True,
                        )
                # normalize: y into y_sb[hp] block j, bf16
                ypv = yp[:, 0:130].rearrange("p (h c) -> p h c", h=2)
                rec = smp.tile([128, 2], F32, name="rec")
                nc.vector.reciprocal(rec[:], ypv[:, :, 64])
                dst = (
                    y_sb[hp][:, 128 * j : 128 * j + 128]
                    .rearrange("p (h c) -> p h c", h=2)
                )
                nc.vector.tensor_mul(
                    dst,
                    ypv[:, :, 0:64],
                    rec[:].unsqueeze(2).broadcast_to([128, 2, 64]),
                )

            def emit_transpose_group(hp, jg, tp_pool, tp_tag):
                """Transpose y_sb[hp] tiles 4jg..4jg+3 into yT_cat via PE
                matmul with identity; copy PSUM->SBUF bf16 on ACT."""
                tp = tp_pool.tile([128, 512], F32, name="ytp_t", tag=tp_tag)
                for jj in range(4):
                    j = 4 * jg + jj
                    nc.tensor.matmul(
                        tp[:, 128 * jj : 128 * jj + 128],
                        y_sb[hp][:, 128 * j : 128 * j + 128],
                        ident_sb[:],
                        start=True,
                        stop=True,
                        skip_group_check=True,
                    )
                nc.vector.tensor_copy(
                    yT_cat[:, T * hp + 512 * jg : T * hp + 512 * jg + 512], tp[:]
                )

            # ---- scores/QKV scope ----
            with (
                tc.tile_pool(name="sps", bufs=2, space="PSUM") as sps,
                tc.tile_pool(name="xw", bufs=1) as xw,
                tc.tile_pool(name="mmps", bufs=2, space="PSUM") as mmps,
            ):
                xh_sb = xw.tile([128, 8 * 2048], E4, name="xh")
                xl_sb = xw.tile([128, 8 * 2048], E4, name="xl")
                wq_sb = [xw.tile([128, 2 * 8 * 128], E4, name=f"wq{hp}")
                         for hp in range(2)]
                wk_sb = [xw.tile([128, 2 * 8 * 128], E4, name=f"wk{hp}")
                         for hp in range(2)]
                wv_sb = xw.tile([128, 2 * 8 * 256], E4, name="wv")

                # x + weights on SP HWDGE and SWDGE only: the ACT queue must
                # stay clear, DMA dispatches there would serialize with exp
                # on the ACT sequencer.
                xh_r = xh_sb.rearrange("p (c n) -> p c n", c=8)
                xl_r = xl_sb.rearrange("p (c n) -> p c n", c=8)
                xhd_r = xh_d[:].rearrange("p (c n) -> p c n", c=8)
                xld_r = xl_d[:].rearrange("p (c n) -> p c n", c=8)
                # all of x on SP in consumption order, few big transfers
                # (each DMACopy pays ~625ns HWDGE generation serially)
                nc.sync.dma_start(wq_sb[0][:], wq_d[0])
                nc.sync.dma_start(xh_sb[:, 0 : 4 * 2048], xh_d[:, 0 : 4 * 2048])
                nc.sync.dma_start(xh_sb[:, 4 * 2048 :], xh_d[:, 4 * 2048 :])
                nc.sync.dma_start(wk_sb[0][:], wk_d[0])
                nc.sync.dma_start(xl_sb[:, 0 : 4 * 2048], xl_d[:, 0 : 4 * 2048])
                nc.sync.dma_start(xl_sb[:, 4 * 2048 :], xl_d[:, 4 * 2048 :])
                nc.sync.dma_start(wq_sb[1][:], wq_d[1])
                nc.sync.dma_start(wk_sb[1][:], wk_d[1])
                nc.sync.dma_start(wv_sb[:], wv_d[:])

                def emit_qk_group(hp, kind, tq):
                    """q or k for head pair hp, 512 t-cols starting 512*tq.
                    Compensated fp8: xh*wh + xh*wl + xl*wh, DR chunk pairs.
                    One DVE fp8 copy + one Pool bf16 copy per group."""
                    w_sb = wq_sb if kind == 0 else wk_sb
                    d8 = q8[hp] if kind == 0 else k8[hp]
                    d16 = q16[hp] if kind == 0 else k16[hp]
                    pt = mmps.tile([128, 512], F32, name="qkps", tag="qk")
                    for half in range(2):
                        n0 = 512 * tq + 256 * half
                        first = True
                        for xs, wl_ in ((xh_r, 0), (xh_r, 1), (xl_r, 0)):
                            wr = w_sb[hp][:, 1024 * wl_ : 1024 * wl_ + 1024].rearrange(
                                "p (c m) -> p c m", c=8)
                            for cp in range(4):
                                nc.tensor.matmul(
                                    pt[:, 256 * half : 256 * half + 256],
                                    wr[:, 2 * cp : 2 * cp + 2, :],
                                    xs[:, 2 * cp : 2 * cp + 2, n0 : n0 + 256],
                                    start=first,
                                    stop=(wl_ == 0 and xs is xl_r and cp == 3),
                                    perf_mode=mybir.MatmulPerfMode.DoubleRow,
                                )
                                first = False
                    n0 = 512 * tq
                    # Pool cannot read PSUM: bf16 from PSUM on DVE, then
                    # fp8 from the bf16 copy on Pool (SBUF->SBUF)
                    nc.vector.tensor_copy(d16[:, n0 : n0 + 512], pt[:])
                    nc.gpsimd.tensor_copy(
                        d8[:, n0 : n0 + 512], d16[:, n0 : n0 + 512]
                    )

                def emit_v_t(t):
                    pv = mmps.tile([128, 512], F32, name="vps", tag="qk")[:, 0:256]
                    first = True
                    for xs, wl_ in ((xh_r, 0), (xh_r, 1), (xl_r, 0)):
                        wr = wv_sb[:, 2048 * wl_ : 2048 * wl_ + 2048].rearrange(
                            "p (c m) -> p c m", c=8)
                        for cp in range(4):
                            nc.tensor.matmul(
                                pv[:],
                                xs[:, 2 * cp : 2 * cp + 2, 128 * t : 128 * t + 128],
                                wr[:, 2 * cp : 2 * cp + 2, :],
                                start=first,
                                stop=(wl_ == 0 and xs is xl_r and cp == 3),
                                perf_mode=mybir.MatmulPerfMode.DoubleRow,
                            )
                            first = False
                    # v = pv / WS, bf16
                    nc.vector.tensor_scalar_mul(
                        v_sb[t].rearrange("p (h c) -> p h c", h=4)[:, :, 0:64],
                        pv[:].rearrange("p (h c) -> p h c", h=4),
                        1.0 / WS,
                    )

                # PE warm-up on the mask tile while input DMAs land.
                warm = sps.tile([128, 1024], F32, name="warm", tag="s")
                for i in range(16):
                    nc.tensor.matmul(
                        warm[:, 0:128], mask_sb[:], mask_sb[:],
                        start=True, stop=True,
                    )
                # Phase A: q(hp0) groups 0,1 then k(hp0) group 0, so the
                # first score parts (tq < 1024) can start before all of q
                # is projected.
                emit_qk_group(0, 0, 0)
                emit_qk_group(0, 0, 1)
                emit_qk_group(0, 1, 0)
                for hp in range(2):
                    nc.gpsimd.dma_start(wpT_sb[hp][:], wpT_d[hp])
                # Phase B head: interleave remaining q groups with the
                # early score parts of tiles 0..3
                emit_scores_tk(0, 0, sps, only_part=0)
                emit_qk_group(0, 0, 2)
                emit_scores_tk(0, 1, sps, only_part=0)
                emit_qk_group(0, 0, 3)
                emit_scores_tk(0, 2, sps, only_part=0)
                emit_scores_tk(0, 0, sps, only_part=1)
                emit_scores_tk(0, 3, sps, only_part=0)
                emit_scores_tk(0, 1, sps, only_part=1)
                emit_qk_group(1, 0, 0)
                emit_scores_tk(0, 2, sps, only_part=1)
                emit_scores_tk(0, 3, sps, only_part=1)
                for g in range(1, 4):
                    emit_qk_group(0, 1, g)
                    emit_scores_tk(0, 4 * g, sps)
                    emit_scores_tk(0, 4 * g + 1, sps)
                    emit_qk_group(1, 0, g)
                    emit_scores_tk(0, 4 * g + 2, sps)
                    emit_scores_tk(0, 4 * g + 3, sps)
                # Phase C: k(hp1) + scores h1 + v
                for g in range(4):
                    emit_qk_group(1, 1, g)
                    emit_scores_tk(1, 4 * g, sps)
                    emit_v_t(4 * g)
                    emit_scores_tk(1, 4 * g + 1, sps)
                    emit_v_t(4 * g + 1)
                    emit_scores_tk(1, 4 * g + 2, sps)
                    emit_v_t(4 * g + 2)
                    emit_scores_tk(1, 4 * g + 3, sps)
                    emit_v_t(4 * g + 3)
                # Phase D: scores h2 + AV pair hp0 + transposes hp0,
                # score parts split around AV so ACT is fed evenly
                for g in range(4):
                    for i in range(4):
                        tk = 4 * g + i
                        emit_scores_tk(2, tk, sps, only_part=0)
                        emit_av_pair(0, tk)
                        emit_scores_tk(2, tk, sps, only_part=1)
                    emit_transpose_group(0, g, mmps, "qk")

            # ---- tail scope: scores h3 + AV hp1 + proj ----
            with (
                tc.tile_pool(name="sps2", bufs=2, space="PSUM") as sps2,
                tc.tile_pool(name="pps", bufs=2, space="PSUM") as pps,
                tc.tile_pool(name="outs", bufs=8) as outs,
            ):
                def emit_proj_t(t):
                    pps_t = {}
                    for n in range(2):
                        pp = pps.tile([128, 512], F32, name="pp", tag="p")
                        pps_t[n] = pp
                        nc.tensor.matmul(
                            pp[:],
                            yT_cat[:, 128 * t : 128 * t + 128],
                            wpT_sb[0][:, 512 * n : 512 * n + 512],
                            start=True,
                            stop=False,
                            skip_group_check=True,
                        )
                    for n in range(2):
                        pp = pps_t[n]
                        nc.tensor.matmul(
                            pp[:],
                            yT_cat[:, T + 128 * t : T + 128 * t + 128],
                            wpT_sb[1][:, 512 * n : 512 * n + 512],
                            start=False,
                            stop=True,
                            skip_group_check=True,
                        )
                        ot = outs.tile([128, 512], BF16, name="ot")
                        if n == 1 and t >= 8:
                            nc.scalar.copy(ot[:], pp[:])
                        else:
                            nc.vector.tensor_copy(ot[:], pp[:])
                        eng = nc.gpsimd if (t >= 14 and n == 1) else nc.sync
                        eng.dma_start(
                            out_d[
                                128 * t : 128 * t + 128,
                                512 * n : 512 * n + 512,
                            ],
                            ot[:],
                        )

                # Phase E: scores h3 (512-col psum chunks) + AV hp1 +
                # transposes hp1 + early proj pairs
                for g in range(4):
                    for i in range(4):
                        j = 4 * g + i
                        emit_scores_tk(3, j, sps2, part_w=1024, only_part=0)
                        emit_av_pair(1, j)
                        emit_scores_tk(3, j, sps2, part_w=1024, only_part=1)
                        # per-tile transpose: shortens the post-exp tail
                        tp = yps.tile([128, 512], F32, name="ytp_t", tag="y")
                        nc.tensor.matmul(
                            tp[:, 0:128],
                            y_sb[1][:, 128 * j : 128 * j + 128],
                            ident_sb[:],
                            start=True,
                            stop=True,
                            skip_group_check=True,
                        )
                        nc.vector.tensor_copy(
                            yT_cat[:, T + 128 * j : T + 128 * j + 128],
                            tp[:, 0:128],
                        )
                        if g >= 1:
                            emit_proj_t(4 * (g - 1) + i)
                # Phase F: remaining proj
                for t in range(12, 16):
                    emit_proj_t(t)

    nc.finalize()
    return nc


def _get_nc():
    global _cached_nc
    if _cached_nc is None:
        _cached_nc = _build()
    return _cached_nc


def kernel(x, Wq, Wk, Wv, Wp, bp):
    global last_results
    x = np.asarray(x, dtype=np.float32)
    Wq = np.asarray(Wq, dtype=np.float32)
    Wk = np.asarray(Wk, dtype=np.float32)
    Wv = np.asarray(Wv, dtype=np.float32)
    Wp = np.asarray(Wp, dtype=np.float32)
    bp = np.asarray(bp, dtype=np.float32)

    E4NP = ml_dtypes.float8_e4m3
    WpT = np.ascontiguousarray(Wp.T)  # [C_in(features), C_out]
    mask01 = np.triu(np.ones((128, 128), dtype=np.float32)).astype(ml_dtypes.bfloat16)
    ident = np.eye(128, dtype=np.float32).astype(ml_dtypes.bfloat16)

    def chunked(w):
        # [C, m] -> [128, 8*m]: c-chunk c at cols [m*c : m*(c+1)]
        m = w.shape[1]
        return np.ascontiguousarray(
            w.reshape(8, 128, m).transpose(1, 0, 2).reshape(128, 8 * m)
        )

    def hilo(a):
        hi = a.astype(E4NP)
        lo = (a - hi.astype(np.float32)).astype(E4NP)
        return hi, lo

    xT_by_batch = [np.ascontiguousarray(x[b].T) for b in range(B)]
    xhl_by_batch = [hilo(chunked(xT_by_batch[b])) for b in range(B)]

    in_maps = []
    for core in range(N_CORES):
        b, g = core // 4, core % 4
        h0 = HPC * g
        def wq_pair(W):
            res = []
            for hp in range(2):
                cat = np.concatenate(
                    [W[h0 + 2 * hp], W[h0 + 2 * hp + 1]], axis=1
                ) * WS
                h_, l_ = hilo(chunked(cat))
                res.append(np.concatenate([h_, l_], axis=1))
            return np.stack(res)
        wq_p = wq_pair(Wq)
        wk_p = wq_pair(Wk)
        wv_cat = np.concatenate([Wv[h0 + j] for j in range(HPC)], axis=1) * WS
        wvh, wvl = hilo(chunked(wv_cat))
        wv_p = np.concatenate([wvh, wvl], axis=1)
        wpT_p = np.ascontiguousarray(
            WpT[256 * g : 256 * (g + 1)].reshape(2, 128, C)
        ).astype(ml_dtypes.bfloat16)
        xh, xl = xhl_by_batch[b]
        in_maps.append(
            {
                "xh": xh, "xl": xl,
                "wq": wq_p, "wk": wk_p, "wv": wv_p,
                "wpT": wpT_p,
                "mask": mask01,
                "ident": ident,
            }
        )

    nc = _get_nc()
    kwargs = {}
    if os.environ.get("KERNEL_TRACE", "0") == "1":
        kwargs = dict(trace=True, trace_cores=list(range(N_CORES)),
                      stitch_traces=True)
    try:
        res = run_bass_kernel_spmd(
            nc, in_maps, core_ids=list(range(N_CORES)), **kwargs
        )
    except ModuleNotFoundError:
        res = run_bass_kernel_spmd(nc, in_maps, core_ids=list(range(N_CORES)))
    last_results = res

    out = np.zeros((B, T, C), dtype=np.float32)
    for core in range(N_CORES):
        b = core // 4
        out[b] += res.results[core]["out"].astype(np.float32)
    out += bp[None, None, :]
    return out


# revision 31
# speedup vs baseline: 1.1076x; 1.0087x over previous
"""Multi-head causal attention (B=2, T=2048, C=1024, H=16, S=64) on 8 TRN2 cores.

Sharding: core i handles batch b = i//4 and head group g = i%4 (4 heads each).
Each core computes a partial output projection (its heads' contribution to the
full [T, C] output); the host sums the 4 partials per batch and adds the bias.

V2 dataflow (cost model: matmul time = out_free_cols x cycles(moving dtype),
fp8e4 DoubleRow = 0.5 cycles/col):
  - QKV projections: error-compensated fp8 (x = xh+xl, W*32 = wh+wl; terms
    xh*wh + xh*wl + xl*wh via DoubleRow chunk pairs; xl*wl dropped).
    12 DR matmuls replace 8 bf16 matmuls per tile: 25% fewer PE cycles.
  - Scores: q,k quantized to fp8; off-diagonal tiles via zero-padded
    DoubleRow (stationary = (k_tile, zeros), moving = q twice, stride-0) at
    0.5 cycles/col. Diagonal tiles in bf16 for accuracy (softmax spike).
  - p = exp(s * 0.125/1024) on ACT (weights carry x32 scale per side).
  - AV reoriented: stationary = p tile [u,tq], moving = v|1 [u,65]; out
    y[tq, 65] accumulated over u tiles in PSUM (col 64 = denominator).
  - Normalize y by 1/d per (pair, tq-tile) on DVE (stride-0 broadcast mul),
    transpose y -> yT via PE matmul against identity.
  - Output projection bf16 (stationary yT, moving WpT), accumulate head
    pairs in PSUM.
"""

import os
import math
import numpy as np
import ml_dtypes

import concourse.bacc as bacc
import concourse.mybir as mybir
import concourse.tile as tile
from concourse.bass_utils import run_bass_kernel_spmd

F32 = mybir.dt.float32
BF16 = mybir.dt.bfloat16
E4 = mybir.dt.float8e4

B, T, C, H, S = 2, 2048, 1024, 16, 64
HPC = 4          # heads per core
N_CORES = 8
NC_T = T // 128  # 16 t-tiles of 128
WS = 32.0        # weight scale folded into fp8 weights

# p storage offsets: tile tk spans tq in [128*tk, 2048)
SPAN = [T - 128 * tk for tk in range(NC_T)]
OFF = [0] * NC_T
for _tk in range(1, NC_T):
    OFF[_tk] = OFF[_tk - 1] + SPAN[_tk - 1]
ATT_W = OFF[-1] + SPAN[-1]  # 17408

_cached_nc = None
last_results = None  # BassKernelResults of the most recent run (for test harness)


def _build():
    nc = bacc.Bacc("TRN2", target_bir_lowering=False)

    # fp8 hi/lo inputs, chunk-major so each DMA is contiguous per partition.
    xh_d = nc.dram_tensor("xh", [128, 8 * 2048], E4, kind="ExternalInput")
    xl_d = nc.dram_tensor("xl", [128, 8 * 2048], E4, kind="ExternalInput")
    wq_d = nc.dram_tensor("wq", [2, 128, 2 * 8 * 128], E4, kind="ExternalInput")
    wk_d = nc.dram_tensor("wk", [2, 128, 2 * 8 * 128], E4, kind="ExternalInput")
    wv_d = nc.dram_tensor("wv", [128, 2 * 8 * 256], E4, kind="ExternalInput")
    wpT_d = nc.dram_tensor("wpT", [2, 128, C], BF16, kind="ExternalInput")
    mask_d = nc.dram_tensor("mask", [128, 128], BF16, kind="ExternalInput")
    ident_d = nc.dram_tensor("ident", [128, 128], BF16, kind="ExternalInput")
    out_d = nc.dram_tensor("out", [T, C], BF16, kind="ExternalOutput")

    with tile.TileContext(nc) as tc:
        with (
            tc.tile_pool(name="const", bufs=1) as constp,
            tc.tile_pool(name="qk8", bufs=1) as qk8p,
            tc.tile_pool(name="qk16", bufs=1) as qk16p,
            tc.tile_pool(name="vsb", bufs=1) as vp,
            tc.tile_pool(name="ysb", bufs=1) as ysbp,
            tc.tile_pool(name="yT", bufs=1) as ytcp,
            tc.tile_pool(name="attp", bufs=1) as attp,
            tc.tile_pool(name="sm", bufs=2) as smp,
            tc.tile_pool(name="ypsum", bufs=2, space="PSUM") as yps,
        ):
            # persistent tiles
            mask_sb = constp.tile([128, 128], BF16, name="mask_sb")
            nc.gpsimd.dma_start(mask_sb[:], mask_d[:])
            ident_sb = constp.tile([128, 128], BF16, name="ident_sb")
            nc.gpsimd.dma_start(ident_sb[:], ident_d[:])

            # fp8 q/k: q gets 128 zero-pad cols (moving side of the
            # zero-padded DoubleRow score matmuls; q writes complete early
            # so the wide moving AP causes no late false deps)
            q8 = [qk8p.tile([128, T + 128], E4, name=f"q8_{hp}") for hp in range(2)]
            k8 = [qk8p.tile([128, T], E4, name=f"k8_{hp}") for hp in range(2)]
            for hp in range(2):
                nc.vector.memset(q8[hp][:, T : T + 128], 0.0)
            # bf16 q/k for the diagonal score tiles
            q16 = [qk16p.tile([128, T], BF16, name=f"q16_{hp}") for hp in range(2)]
            k16 = [qk16p.tile([128, T], BF16, name=f"k16_{hp}") for hp in range(2)]

            # v tiles: [128, 4*65] bf16; head h in cols 65h..65h+63, col 65h+64 = 1
            v_sb = [vp.tile([128, 4 * 65], BF16, name=f"v{t}") for t in range(NC_T)]
            for t in range(NC_T):
                ones_ap = v_sb[t].rearrange("p (h c) -> p h c", h=4)[:, :, 64]
                nc.vector.memset(ones_ap, 1.0)

            # normalized y staging [tq, (h_even|h_odd)] per pair, per tq tile
            y_sb = [ysbp.tile([128, NC_T * 128], BF16, name=f"ysb{hp}")
                    for hp in range(2)]
            # concatenated yT for proj: dim1 = hp
            yT_cat = ytcp.tile([128, 2 * T], BF16, name="yT_cat")

            # p buffers (3-deep head pipeline: exp(h+2) must not wait on
            # AV(h) finishing)
            att_buf = [attp.tile([128, ATT_W], BF16, name=f"attb{i}")
                       for i in range(3)]
            BUF_OF = [0, 1, 2, 0]  # head -> p buffer

            wpT_sb = [constp.tile([128, C], BF16, name=f"wpT{hp}")
                      for hp in range(2)]

            # Schraudolph fast-exp constants (bf16 bit trick):
            # bits_i16 = A*z + B with z = s_psum * 0.125/WS^2
            SCH_A = 184.66496 * 0.125 / (WS * WS)
            SCH_B = 16252.0
            exp_ctr = [0]

            def emit_exp(dst, src):
                exp_ctr[0] += 1
                if exp_ctr[0] % 5 == 0:
                    # DVE fast-exp: affine into int16, bitcast to bf16
                    nc.vector.tensor_scalar(
                        dst.bitcast(mybir.dt.int16),
                        src,
                        SCH_A,
                        SCH_B,
                        mybir.AluOpType.mult,
                        mybir.AluOpType.add,
                    )
                else:
                    nc.scalar.activation(
                        dst, src,
                        mybir.ActivationFunctionType.Exp,
                        scale=0.125 / (WS * WS),
                    )

            def emit_scores_tk(h, tk, sps_pool, part_w=1024, only_part=None):
                """Scores for head h, k-tile tk: diag 128 cols in bf16 +
                off-diag in zero-padded fp8 DR chunks; exp into att_buf."""
                hp, half = h // 2, h % 2
                r0 = 64 * half
                ab = att_buf[BUF_OF[h]]
                span = SPAN[tk]
                kt16 = k16[hp][r0 : r0 + 64, 128 * tk : 128 * tk + 128]
                # stationary fp8: (k tile, k tile) via stride-0 broadcast;
                # the moving q side supplies (q cols, zeros) so the second
                # k contribution is k.T @ 0 = 0.
                kt8 = (
                    k8[hp][r0 : r0 + 64, 128 * tk : 128 * tk + 128]
                    .unsqueeze(1)
                    .broadcast_to([64, 2, 128])
                )
                for part in range(math.ceil(span / part_w)):
                    if only_part is not None and part != only_part:
                        continue
                    pspan = min(part_w, span - part_w * part)
                    pt = sps_pool.tile([128, part_w], F32, name="sps_t", tag="s")
                    c0 = 0
                    if part == 0:
                        # diagonal block in bf16 (K=64)
                        nc.tensor.matmul(
                            pt[:, 0:128],
                            kt16,
                            q16[hp][r0 : r0 + 64, 128 * tk : 128 * tk + 128],
                            start=True,
                            stop=True,
                        )
                        c0 = 128
                    while c0 < pspan:
                        n = min(128, pspan - c0)
                        tq0 = 128 * tk + part_w * part + c0
                        # moving: (q cols tq0.., zero pad at col T), built by
                        # 128-col rechunking + stride slicing
                        nch = (T + 128 - tq0) // 128
                        qmov = (
                            q8[hp][r0 : r0 + 64, tq0 : T + 128]
                            .rearrange("p (x m) -> p x m", x=nch)[:, 0 :: max(nch - 1, 1), :]
                        )
                        if nch == 1:
                            qmov = qmov.broadcast_to([64, 2, n])
                        nc.tensor.matmul(
                            pt[:, c0 : c0 + n],
                            kt8,
                            qmov[:, :, 0:n] if n != 128 else qmov,
                            start=True,
                            stop=True,
                            perf_mode=mybir.MatmulPerfMode.DoubleRow,
                        )
                        c0 += n
                    dst = ab[
                        :, OFF[tk] + part_w * part : OFF[tk] + part_w * part + pspan
                    ]
                    emit_exp(dst, pt[:, 0:pspan])
                # mask the diagonal block (first 128 cols of this tk tile)
                diag = ab[:, OFF[tk] : OFF[tk] + 128]
                nc.gpsimd.tensor_mul(diag, diag, mask_sb[:])

            def emit_av_pair(hp, j):
                """y[tq, 65] for both heads of pair hp, tq-tile j, then
                normalize into y_sb[hp] block j."""
                yp = yps.tile([128, 512], F32, name="yps_t", tag="y")
                for half in range(2):
                    h = 2 * hp + half
                    ab = att_buf[BUF_OF[h]]
                    for tk in range(j + 1):
                        ptile = ab[:, OFF[tk] + 128 * (j - tk) : OFF[tk] + 128 * (j - tk) + 128]
                        nc.tensor.matmul(
                            yp[:, 65 * half : 65 * half + 65],
                            ptile,
                            v_sb[tk][:, 65 * h : 65 * h + 65],
                            start=(tk == 0),
                            stop=(tk == j),
                            skip_group_check=True,
                        )
                # normalize: y into y_sb[hp] block j, bf16
                ypv = yp[:, 0:130].rearrange("p (h c) -> p h c", h=2)
                rec = smp.tile([128, 2], F32, name="rec")
                nc.vector.reciprocal(rec[:], ypv[:, :, 64])
                dst = (
                    y_sb[hp][:, 128 * j : 128 * j + 128]
                    .rearrange("p (h c) -> p h c", h=2)
                )
                nc.vector.tensor_mul(
                    dst,
                    ypv[:, :, 0:64],
                    rec[:].unsqueeze(2).broadcast_to([128, 2, 64]),
                )

            def emit_transpose_group(hp, jg, tp_pool, tp_tag):
                """Transpose y_sb[hp] tiles 4jg..4jg+3 into yT_cat via PE
                matmul with identity; copy PSUM->SBUF bf16 on ACT."""
                tp = tp_pool.tile([128, 512], F32, name="ytp_t", tag=tp_tag)
                for jj in range(4):
                    j = 4 * jg + jj
                    nc.tensor.matmul(
                        tp[:, 128 * jj : 128 * jj + 128],
                        y_sb[hp][:, 128 * j : 128 * j + 128],
                        ident_sb[:],
                        start=True,
                        stop=True,
                        skip_group_check=True,
                    )
                nc.vector.tensor_copy(
                    yT_cat[:, T * hp + 512 * jg : T * hp + 512 * jg + 512], tp[:]
                )

            # ---- scores/QKV scope ----
            with (
                tc.tile_pool(name="sps", bufs=2, space="PSUM") as sps,
                tc.tile_pool(name="xw", bufs=1) as xw,
                tc.tile_pool(name="mmps", bufs=2, space="PSUM") as mmps,
            ):
                xh_sb = xw.tile([128, 8 * 2048], E4, name="xh")
                xl_sb = xw.tile([128, 8 * 2048], E4, name="xl")
                wq_sb = [xw.tile([128, 2 * 8 * 128], E4, name=f"wq{hp}")
                         for hp in range(2)]
                wk_sb = [xw.tile([128, 2 * 8 * 128], E4, name=f"wk{hp}")
                         for hp in range(2)]
                wv_sb = xw.tile([128, 2 * 8 * 256], E4, name="wv")

                # x + weights on SP HWDGE and SWDGE only: the ACT queue must
                # stay clear, DMA dispatches there would serialize with exp
                # on the ACT sequencer.
                xh_r = xh_sb.rearrange("p (c n) -> p c n", c=8)
                xl_r = xl_sb.rearrange("p (c n) -> p c n", c=8)
                xhd_r = xh_d[:].rearrange("p (c n) -> p c n", c=8)
                xld_r = xl_d[:].rearrange("p (c n) -> p c n", c=8)
                # all of x on SP in consumption order, few big transfers
                # (each DMACopy pays ~625ns HWDGE generation serially)
                nc.sync.dma_start(wq_sb[0][:], wq_d[0])
                nc.sync.dma_start(xh_sb[:, 0 : 4 * 2048], xh_d[:, 0 : 4 * 2048])
                nc.sync.dma_start(xh_sb[:, 4 * 2048 :], xh_d[:, 4 * 2048 :])
                nc.sync.dma_start(wk_sb[0][:], wk_d[0])
                nc.sync.dma_start(xl_sb[:, 0 : 4 * 2048], xl_d[:, 0 : 4 * 2048])
                nc.sync.dma_start(xl_sb[:, 4 * 2048 :], xl_d[:, 4 * 2048 :])
                nc.sync.dma_start(wq_sb[1][:], wq_d[1])
                nc.sync.dma_start(wk_sb[1][:], wk_d[1])
                nc.sync.dma_start(wv_sb[:], wv_d[:])

                def emit_qk_group(hp, kind, tq):
                    """q or k for head pair hp, 512 t-cols starting 512*tq.
                    Compensated fp8: xh*wh + xh*wl + xl*wh, DR chunk pairs.
                    One DVE fp8 copy + one Pool bf16 copy per group."""
                    w_sb = wq_sb if kind == 0 else wk_sb
                    d8 = q8[hp] if kind == 0 else k8[hp]
                    d16 = q16[hp] if kind == 0 else k16[hp]
                    pt = mmps.tile([128, 512], F32, name="qkps", tag="qk")
                    for half in range(2):
                        n0 = 512 * tq + 256 * half
                        first = True
                        for xs, wl_ in ((xh_r, 0), (xh_r, 1), (xl_r, 0)):
                            wr = w_sb[hp][:, 1024 * wl_ : 1024 * wl_ + 1024].rearrange(
                                "p (c m) -> p c m", c=8)
                            for cp in range(4):
                                nc.tensor.matmul(
                                    pt[:, 256 * half : 256 * half + 256],
                                    wr[:, 2 * cp : 2 * cp + 2, :],
                                    xs[:, 2 * cp : 2 * cp + 2, n0 : n0 + 256],
                                    start=first,
                                    stop=(wl_ == 0 and xs is xl_r and cp == 3),
                                    perf_mode=mybir.MatmulPerfMode.DoubleRow,
                                )
                                first = False
                    n0 = 512 * tq
                    # Pool cannot read PSUM: bf16 from PSUM on DVE, then
                    # fp8 from the bf16 copy on Pool (SBUF->SBUF)
                    nc.vector.tensor_copy(d16[:, n0 : n0 + 512], pt[:])
                    nc.gpsimd.tensor_copy(
                        d8[:, n0 : n0 + 512], d16[:, n0 : n0 + 512]
                    )

                def emit_v_t(t):
                    pv = mmps.tile([128, 512], F32, name="vps", tag="qk")[:, 0:256]
                    first = True
                    for xs, wl_ in ((xh_r, 0), (xh_r, 1), (xl_r, 0)):
                        wr = wv_sb[:, 2048 * wl_ : 2048 * wl_ + 2048].rearrange(
                            "p (c m) -> p c m", c=8)
                        for cp in range(4):
                            nc.tensor.matmul(
                                pv[:],
                                xs[:, 2 * cp : 2 * cp + 2, 128 * t : 128 * t + 128],
                                wr[:, 2 * cp : 2 * cp + 2, :],
                                start=first,
                                stop=(wl_ == 0 and xs is xl_r and cp == 3),
                                perf_mode=mybir.MatmulPerfMode.DoubleRow,
                            )
                            first = False
                    # v = pv / WS, bf16
                    nc.vector.tensor_scalar_mul(
                        v_sb[t].rearrange("p (h c) -> p h c", h=4)[:, :, 0:64],
                        pv[:].rearrange("p (h c) -> p h c", h=4),
                        1.0 / WS,
                    )

                # PE warm-up on the mask tile while input DMAs land.
                warm = sps.tile([128, 1024], F32, name="warm", tag="s")
                for i in range(13):
                    nc.tensor.matmul(
                        warm[:, 0:128], mask_sb[:], mask_sb[:],
                        start=True, stop=True,
                    )
                # Phase A: q(hp0) groups 0,1 then k(hp0) group 0, so the
                # first score parts (tq < 1024) can start before all of q
                # is projected.
                emit_qk_group(0, 0, 0)
                emit_qk_group(0, 0, 1)
                emit_qk_group(0, 1, 0)
                for hp in range(2):
                    nc.gpsimd.dma_start(wpT_sb[hp][:], wpT_d[hp])
                # Phase B head: interleave remaining q groups with the
                # early score parts of tiles 0..3
                emit_scores_tk(0, 0, sps, only_part=0)
                emit_qk_group(0, 0, 2)
                emit_scores_tk(0, 1, sps, only_part=0)
                emit_qk_group(0, 0, 3)
                emit_scores_tk(0, 2, sps, only_part=0)
                emit_scores_tk(0, 0, sps, only_part=1)
                emit_scores_tk(0, 3, sps, only_part=0)
                emit_scores_tk(0, 1, sps, only_part=1)
                emit_qk_group(1, 0, 0)
                emit_scores_tk(0, 2, sps, only_part=1)
                emit_scores_tk(0, 3, sps, only_part=1)
                for g in range(1, 4):
                    emit_qk_group(0, 1, g)
                    emit_scores_tk(0, 4 * g, sps)
                    emit_scores_tk(0, 4 * g + 1, sps)
                    emit_qk_group(1, 0, g)
                    emit_scores_tk(0, 4 * g + 2, sps)
                    emit_scores_tk(0, 4 * g + 3, sps)
                # Phase C: k(hp1) + scores h1 + v
                for g in range(4):
                    emit_qk_group(1, 1, g)
                    emit_scores_tk(1, 4 * g, sps)
                    emit_v_t(4 * g)
                    emit_scores_tk(1, 4 * g + 1, sps)
                    emit_v_t(4 * g + 1)
                    emit_scores_tk(1, 4 * g + 2, sps)
                    emit_v_t(4 * g + 2)
                    emit_scores_tk(1, 4 * g + 3, sps)
                    emit_v_t(4 * g + 3)
                # Phase D: scores h2 + AV pair hp0 + transposes hp0,
                # score parts split around AV so ACT is fed evenly
                for g in range(4):
                    for i in range(4):
                        tk = 4 * g + i
                        emit_scores_tk(2, tk, sps, only_part=0)
                        emit_av_pair(0, tk)
                        emit_scores_tk(2, tk, sps, only_part=1)
                    emit_transpose_group(0, g, mmps, "qk")

            # ---- tail scope: scores h3 + AV hp1 + proj ----
            with (
                tc.tile_pool(name="sps2", bufs=2, space="PSUM") as sps2,
                tc.tile_pool(name="pps", bufs=2, space="PSUM") as pps,
                tc.tile_pool(name="outs", bufs=8) as outs,
            ):
                def emit_proj_t(t):
                    pps_t = {}
                    for n in range(2):
                        pp = pps.tile([128, 512], F32, name="pp", tag="p")
                        pps_t[n] = pp
                        nc.tensor.matmul(
                            pp[:],
                            yT_cat[:, 128 * t : 128 * t + 128],
                            wpT_sb[0][:, 512 * n : 512 * n + 512],
                            start=True,
                            stop=False,
                            skip_group_check=True,
                        )
                    for n in range(2):
                        pp = pps_t[n]
                        nc.tensor.matmul(
                            pp[:],
                            yT_cat[:, T + 128 * t : T + 128 * t + 128],
                            wpT_sb[1][:, 512 * n : 512 * n + 512],
                            start=False,
                            stop=True,
                            skip_group_check=True,
                        )
                        ot = outs.tile([128, 512], BF16, name="ot")
                        if n == 1 and t >= 8:
                            nc.scalar.copy(ot[:], pp[:])
                        else:
                            nc.vector.tensor_copy(ot[:], pp[:])
                        eng = nc.gpsimd if (t >= 14 and n == 1) else nc.sync
                        eng.dma_start(
                            out_d[
                                128 * t : 128 * t + 128,
                                512 * n : 512 * n + 512,
                            ],
                            ot[:],
                        )

                # Phase E: scores h3 (512-col psum chunks) + AV hp1 +
                # transposes hp1 + early proj pairs
                for g in range(4):
                    for i in range(4):
                        j = 4 * g + i
                        emit_scores_tk(3, j, sps2, part_w=1024, only_part=0)
                        emit_av_pair(1, j)
                        emit_scores_tk(3, j, sps2, part_w=1024, only_part=1)
                        # per-tile transpose: shortens the post-exp tail
                        tp = yps.tile([128, 512], F32, name="ytp_t", tag="y")
                        nc.tensor.matmul(
                            tp[:, 0:128],
                            y_sb[1][:, 128 * j : 128 * j + 128],
                            ident_sb[:],
                            start=True,
                            stop=True,
                            skip_group_check=True,
                        )
                        nc.vector.tensor_copy(
                            yT_cat[:, T + 128 * j : T + 128 * j + 128],
                            tp[:, 0:128],
                        )
                        if g >= 1:
                            emit_proj_t(4 * (g - 1) + i)
                # Phase F: remaining proj
                for t in range(12, 16):
                    emit_proj_t(t)

    nc.finalize()
    return nc


def _get_nc():
    global _cached_nc
    if _cached_nc is None:
        _cached_nc = _build()
    return _cached_nc


def kernel(x, Wq, Wk, Wv, Wp, bp):
    global last_results
    x = np.asarray(x, dtype=np.float32)
    Wq = np.asarray(Wq, dtype=np.float32)
    Wk = np.asarray(Wk, dtype=np.float32)
    Wv = np.asarray(Wv, dtype=np.float32)
    Wp = np.asarray(Wp, dtype=np.float32)
    bp = np.asarray(bp, dtype=np.float32)

    E4NP = ml_dtypes.float8_e4m3
    WpT = np.ascontiguousarray(Wp.T)  # [C_in(features), C_out]
    mask01 = np.triu(np.ones((128, 128), dtype=np.float32)).astype(ml_dtypes.bfloat16)
    ident = np.eye(128, dtype=np.float32).astype(ml_dtypes.bfloat16)

    def chunked(w):
        # [C, m] -> [128, 8*m]: c-chunk c at cols [m*c : m*(c+1)]
        m = w.shape[1]
        return np.ascontiguousarray(
            w.reshape(8, 128, m).transpose(1, 0, 2).reshape(128, 8 * m)
        )

    def hilo(a):
        hi = a.astype(E4NP)
        lo = (a - hi.astype(np.float32)).astype(E4NP)
        return hi, lo

    xT_by_batch = [np.ascontiguousarray(x[b].T) for b in range(B)]
    xhl_by_batch = [hilo(chunked(xT_by_batch[b])) for b in range(B)]

    in_maps = []
    for core in range(N_CORES):
        b, g = core // 4, core % 4
        h0 = HPC * g
        def wq_pair(W):
            res = []
            for hp in range(2):
                cat = np.concatenate(
                    [W[h0 + 2 * hp], W[h0 + 2 * hp + 1]], axis=1
                ) * WS
                h_, l_ = hilo(chunked(cat))
                res.append(np.concatenate([h_, l_], axis=1))
            return np.stack(res)
        wq_p = wq_pair(Wq)
        wk_p = wq_pair(Wk)
        wv_cat = np.concatenate([Wv[h0 + j] for j in range(HPC)], axis=1) * WS
        wvh, wvl = hilo(chunked(wv_cat))
        wv_p = np.concatenate([wvh, wvl], axis=1)
        wpT_p = np.ascontiguousarray(
            WpT[256 * g : 256 * (g + 1)].reshape(2, 128, C)
        ).astype(ml_dtypes.bfloat16)
        xh, xl = xhl_by_batch[b]
        in_maps.append(
            {
                "xh": xh, "xl": xl,
                "wq": wq_p, "wk": wk_p, "wv": wv_p,
                "wpT": wpT_p,
                "mask": mask01,
                "ident": ident,
            }
        )

    nc = _get_nc()
    kwargs = {}
    if os.environ.get("KERNEL_TRACE", "0") == "1":
        kwargs = dict(trace=True, trace_cores=list(range(N_CORES)),
                      stitch_traces=True)
    try:
        res = run_bass_kernel_spmd(
            nc, in_maps, core_ids=list(range(N_CORES)), **kwargs
        )
    except ModuleNotFoundError:
        res = run_bass_kernel_spmd(nc, in_maps, core_ids=list(range(N_CORES)))
    last_results = res

    out = np.zeros((B, T, C), dtype=np.float32)
    for core in range(N_CORES):
        b = core // 4
        out[b] += res.results[core]["out"].astype(np.float32)
    out += bp[None, None, :]
    return out


# revision 37
# speedup vs baseline: 1.1271x; 1.0176x over previous
"""Multi-head causal attention (B=2, T=2048, C=1024, H=16, S=64) on 8 TRN2 cores.

Sharding: core i handles batch b = i//4 and head group g = i%4 (4 heads each).
Each core computes a partial output projection (its heads' contribution to the
full [T, C] output); the host sums the 4 partials per batch and adds the bias.

V2 dataflow (cost model: matmul time = out_free_cols x cycles(moving dtype),
fp8e4 DoubleRow = 0.5 cycles/col):
  - QKV projections: error-compensated fp8 (x = xh+xl, W*32 = wh+wl; terms
    xh*wh + xh*wl + xl*wh via DoubleRow chunk pairs; xl*wl dropped).
    12 DR matmuls replace 8 bf16 matmuls per tile: 25% fewer PE cycles.
  - Scores: q,k quantized to fp8; off-diagonal tiles via zero-padded
    DoubleRow (stationary = (k_tile, zeros), moving = q twice, stride-0) at
    0.5 cycles/col. Diagonal tiles in bf16 for accuracy (softmax spike).
  - p = exp(s * 0.125/1024) on ACT (weights carry x32 scale per side).
  - AV reoriented: stationary = p tile [u,tq], moving = v|1 [u,65]; out
    y[tq, 65] accumulated over u tiles in PSUM (col 64 = denominator).
  - Normalize y by 1/d per (pair, tq-tile) on DVE (stride-0 broadcast mul),
    transpose y -> yT via PE matmul against identity.
  - Output projection bf16 (stationary yT, moving WpT), accumulate head
    pairs in PSUM.
"""

import os
import math
import numpy as np
import ml_dtypes

import concourse.bacc as bacc
import concourse.mybir as mybir
import concourse.tile as tile
from concourse.bass_utils import run_bass_kernel_spmd

F32 = mybir.dt.float32
BF16 = mybir.dt.bfloat16
E4 = mybir.dt.float8e4

B, T, C, H, S = 2, 2048, 1024, 16, 64
HPC = 4          # heads per core
N_CORES = 8
NC_T = T // 128  # 16 t-tiles of 128
WS = 32.0        # weight scale folded into fp8 weights

# p storage offsets: tile tk spans tq in [128*tk, 2048)
SPAN = [T - 128 * tk for tk in range(NC_T)]
OFF = [0] * NC_T
for _tk in range(1, NC_T):
    OFF[_tk] = OFF[_tk - 1] + SPAN[_tk - 1]
ATT_W = OFF[-1] + SPAN[-1]  # 17408

_cached_nc = None
last_results = None  # BassKernelResults of the most recent run (for test harness)


def _build():
    nc = bacc.Bacc("TRN2", target_bir_lowering=False)

    # fp8 hi/lo inputs, chunk-major so each DMA is contiguous per partition.
    xh_d = nc.dram_tensor("xh", [128, 8 * 2048], E4, kind="ExternalInput")
    xl_d = nc.dram_tensor("xl", [128, 8 * 2048], E4, kind="ExternalInput")
    wq_d = nc.dram_tensor("wq", [2, 128, 2 * 8 * 128], E4, kind="ExternalInput")
    wk_d = nc.dram_tensor("wk", [2, 128, 2 * 8 * 128], E4, kind="ExternalInput")
    wv_d = nc.dram_tensor("wv", [128, 2 * 8 * 256], E4, kind="ExternalInput")
    wpT_d = nc.dram_tensor("wpT", [2, 128, C], BF16, kind="ExternalInput")
    mask_d = nc.dram_tensor("mask", [128, 128], BF16, kind="ExternalInput")
    ident_d = nc.dram_tensor("ident", [128, 128], BF16, kind="ExternalInput")
    out_d = nc.dram_tensor("out", [T, C], BF16, kind="ExternalOutput")

    with tile.TileContext(nc) as tc:
        with (
            tc.tile_pool(name="const", bufs=1) as constp,
            tc.tile_pool(name="qk8", bufs=1) as qk8p,
            tc.tile_pool(name="qk16", bufs=1) as qk16p,
            tc.tile_pool(name="vsb", bufs=1) as vp,
            tc.tile_pool(name="ysb", bufs=1) as ysbp,
            tc.tile_pool(name="yT", bufs=1) as ytcp,
            tc.tile_pool(name="attp", bufs=1) as attp,
            tc.tile_pool(name="sm", bufs=2) as smp,
            tc.tile_pool(name="ypsum", bufs=2, space="PSUM") as yps,
        ):
            # persistent tiles
            mask_sb = constp.tile([128, 128], BF16, name="mask_sb")
            nc.gpsimd.dma_start(mask_sb[:], mask_d[:])
            ident_sb = constp.tile([128, 128], BF16, name="ident_sb")
            nc.gpsimd.dma_start(ident_sb[:], ident_d[:])

            # fp8 q/k: q gets 128 zero-pad cols (moving side of the
            # zero-padded DoubleRow score matmuls; q writes complete early
            # so the wide moving AP causes no late false deps)
            q8 = [qk8p.tile([128, T + 128], E4, name=f"q8_{hp}") for hp in range(2)]
            k8 = [qk8p.tile([128, T], E4, name=f"k8_{hp}") for hp in range(2)]
            for hp in range(2):
                nc.vector.memset(q8[hp][:, T : T + 128], 0.0)
            # bf16 q/k for the diagonal score tiles
            q16 = [qk16p.tile([128, T], BF16, name=f"q16_{hp}") for hp in range(2)]
            k16 = [qk16p.tile([128, T], BF16, name=f"k16_{hp}") for hp in range(2)]

            # v tiles: [128, 4*65] bf16; head h in cols 65h..65h+63, col 65h+64 = 1
            v_sb = [vp.tile([128, 4 * 65], BF16, name=f"v{t}") for t in range(NC_T)]
            for t in range(NC_T):
                ones_ap = v_sb[t].rearrange("p (h c) -> p h c", h=4)[:, :, 64]
                nc.vector.memset(ones_ap, 1.0)

            # normalized y staging [tq, (h_even|h_odd)] per pair, per tq tile
            y_sb = [ysbp.tile([128, NC_T * 128], BF16, name=f"ysb{hp}")
                    for hp in range(2)]
            # concatenated yT for proj: dim1 = hp
            yT_cat = ytcp.tile([128, 2 * T], BF16, name="yT_cat")

            # p buffers (3-deep head pipeline: exp(h+2) must not wait on
            # AV(h) finishing)
            att_buf = [attp.tile([128, ATT_W], BF16, name=f"attb{i}")
                       for i in range(3)]
            BUF_OF = [0, 1, 2, 0]  # head -> p buffer

            wpT_sb = [constp.tile([128, C], BF16, name=f"wpT{hp}")
                      for hp in range(2)]

            # Schraudolph fast-exp constants (bf16 bit trick):
            # bits_i16 = A*z + B with z = s_psum * 0.125/WS^2
            SCH_A = 184.66496 * 0.125 / (WS * WS)
            SCH_B = 16252.0
            exp_ctr = [0]

            def emit_exp(dst, src):
                exp_ctr[0] += 1
                if exp_ctr[0] % 5 == 0:
                    # DVE fast-exp: affine into int16, bitcast to bf16
                    nc.vector.tensor_scalar(
                        dst.bitcast(mybir.dt.int16),
                        src,
                        SCH_A,
                        SCH_B,
                        mybir.AluOpType.mult,
                        mybir.AluOpType.add,
                    )
                else:
                    nc.scalar.activation(
                        dst, src,
                        mybir.ActivationFunctionType.Exp,
                        scale=0.125 / (WS * WS),
                    )

            def emit_scores_tk(h, tk, sps_pool, part_w=1024, only_part=None):
                """Scores for head h, k-tile tk: diag 128 cols in bf16 +
                off-diag in zero-padded fp8 DR chunks; exp into att_buf."""
                hp, half = h // 2, h % 2
                r0 = 64 * half
                ab = att_buf[BUF_OF[h]]
                span = SPAN[tk]
                kt16 = k16[hp][r0 : r0 + 64, 128 * tk : 128 * tk + 128]
                # stationary fp8: (k tile, k tile) via stride-0 broadcast;
                # the moving q side supplies (q cols, zeros) so the second
                # k contribution is k.T @ 0 = 0.
                kt8 = (
                    k8[hp][r0 : r0 + 64, 128 * tk : 128 * tk + 128]
                    .unsqueeze(1)
                    .broadcast_to([64, 2, 128])
                )
                for part in range(math.ceil(span / part_w)):
                    if only_part is not None and part != only_part:
                        continue
                    pspan = min(part_w, span - part_w * part)
                    pt = sps_pool.tile([128, part_w], F32, name="sps_t", tag="s")
                    c0 = 0
                    if part == 0:
                        # diagonal block in bf16 (K=64)
                        nc.tensor.matmul(
                            pt[:, 0:128],
                            kt16,
                            q16[hp][r0 : r0 + 64, 128 * tk : 128 * tk + 128],
                            start=True,
                            stop=True,
                        )
                        c0 = 128
                    while c0 < pspan:
                        n = min(128, pspan - c0)
                        tq0 = 128 * tk + part_w * part + c0
                        # moving: (q cols tq0.., zero pad at col T), built by
                        # 128-col rechunking + stride slicing
                        nch = (T + 128 - tq0) // 128
                        qmov = (
                            q8[hp][r0 : r0 + 64, tq0 : T + 128]
                            .rearrange("p (x m) -> p x m", x=nch)[:, 0 :: max(nch - 1, 1), :]
                        )
                        if nch == 1:
                            qmov = qmov.broadcast_to([64, 2, n])
                        nc.tensor.matmul(
                            pt[:, c0 : c0 + n],
                            kt8,
                            qmov[:, :, 0:n] if n != 128 else qmov,
                            start=True,
                            stop=True,
                            perf_mode=mybir.MatmulPerfMode.DoubleRow,
                        )
                        c0 += n
                    dst = ab[
                        :, OFF[tk] + part_w * part : OFF[tk] + part_w * part + pspan
                    ]
                    emit_exp(dst, pt[:, 0:pspan])
                # mask the diagonal block (first 128 cols of this tk tile)
                diag = ab[:, OFF[tk] : OFF[tk] + 128]
                nc.gpsimd.tensor_mul(diag, diag, mask_sb[:])

            def emit_av_pair(hp, j):
                """y[tq, 65] for both heads of pair hp, tq-tile j, then
                normalize into y_sb[hp] block j."""
                yp = yps.tile([128, 512], F32, name="yps_t", tag="y")
                for half in range(2):
                    h = 2 * hp + half
                    ab = att_buf[BUF_OF[h]]
                    for tk in range(j + 1):
                        ptile = ab[:, OFF[tk] + 128 * (j - tk) : OFF[tk] + 128 * (j - tk) + 128]
                        nc.tensor.matmul(
                            yp[:, 65 * half : 65 * half + 65],
                            ptile,
                            v_sb[tk][:, 65 * h : 65 * h + 65],
                            start=(tk == 0),
                            stop=(tk == j),
                            skip_group_check=True,
                        )
                # normalize: y into y_sb[hp] block j, bf16
                ypv = yp[:, 0:130].rearrange("p (h c) -> p h c", h=2)
                rec = smp.tile([128, 2], F32, name="rec")
                nc.vector.reciprocal(rec[:], ypv[:, :, 64])
                dst = (
                    y_sb[hp][:, 128 * j : 128 * j + 128]
                    .rearrange("p (h c) -> p h c", h=2)
                )
                nc.vector.tensor_mul(
                    dst,
                    ypv[:, :, 0:64],
                    rec[:].unsqueeze(2).broadcast_to([128, 2, 64]),
                )

            def emit_transpose_group(hp, jg, tp_pool, tp_tag):
                """Transpose y_sb[hp] tiles 4jg..4jg+3 into yT_cat via PE
                matmul with identity; copy PSUM->SBUF bf16 on ACT."""
                tp = tp_pool.tile([128, 512], F32, name="ytp_t", tag=tp_tag)
                for jj in range(4):
                    j = 4 * jg + jj
                    nc.tensor.matmul(
                        tp[:, 128 * jj : 128 * jj + 128],
                        y_sb[hp][:, 128 * j : 128 * j + 128],
                        ident_sb[:],
                        start=True,
                        stop=True,
                        skip_group_check=True,
                    )
                nc.vector.tensor_copy(
                    yT_cat[:, T * hp + 512 * jg : T * hp + 512 * jg + 512], tp[:]
                )

            # ---- scores/QKV scope ----
            with (
                tc.tile_pool(name="sps", bufs=2, space="PSUM") as sps,
                tc.tile_pool(name="xw", bufs=1) as xw,
                tc.tile_pool(name="mmps", bufs=2, space="PSUM") as mmps,
            ):
                xh_sb = xw.tile([128, 8 * 2048], E4, name="xh")
                xl_sb = xw.tile([128, 8 * 2048], E4, name="xl")
                wq_sb = [xw.tile([128, 2 * 8 * 128], E4, name=f"wq{hp}")
                         for hp in range(2)]
                wk_sb = [xw.tile([128, 2 * 8 * 128], E4, name=f"wk{hp}")
                         for hp in range(2)]
                wv_sb = xw.tile([128, 2 * 8 * 256], E4, name="wv")

                # x + weights on SP HWDGE and SWDGE only: the ACT queue must
                # stay clear, DMA dispatches there would serialize with exp
                # on the ACT sequencer.
                xh_r = xh_sb.rearrange("p (c n) -> p c n", c=8)
                xl_r = xl_sb.rearrange("p (c n) -> p c n", c=8)
                xhd_r = xh_d[:].rearrange("p (c n) -> p c n", c=8)
                xld_r = xl_d[:].rearrange("p (c n) -> p c n", c=8)
                # all of x on SP in consumption order, few big transfers
                # (each DMACopy pays ~625ns HWDGE generation serially)
                nc.sync.dma_start(wq_sb[0][:], wq_d[0])
                nc.sync.dma_start(xh_sb[:, 0 : 4 * 2048], xh_d[:, 0 : 4 * 2048])
                nc.sync.dma_start(xh_sb[:, 4 * 2048 :], xh_d[:, 4 * 2048 :])
                nc.sync.dma_start(wk_sb[0][:], wk_d[0])
                nc.sync.dma_start(xl_sb[:, 0 : 4 * 2048], xl_d[:, 0 : 4 * 2048])
                nc.sync.dma_start(xl_sb[:, 4 * 2048 :], xl_d[:, 4 * 2048 :])
                nc.sync.dma_start(wq_sb[1][:], wq_d[1])
                nc.sync.dma_start(wk_sb[1][:], wk_d[1])
                nc.sync.dma_start(wv_sb[:], wv_d[:])

                def emit_qk_hi(hp, kind, tq):
                    # xh*wh + xh*wl as contiguous chains; returns PSUM tile
                    w_sb = wq_sb if kind == 0 else wk_sb
                    pt = mmps.tile([128, 512], F32, name="qkps", tag="qk")
                    for half in range(2):
                        n0 = 512 * tq + 256 * half
                        first = True
                        for wl_ in (0, 1):
                            wr = w_sb[hp][:, 1024 * wl_ : 1024 * wl_ + 1024].rearrange(
                                "p (c m) -> p c m", c=8)
                            for cp in range(4):
                                nc.tensor.matmul(
                                    pt[:, 256 * half : 256 * half + 256],
                                    wr[:, 2 * cp : 2 * cp + 2, :],
                                    xh_r[:, 2 * cp : 2 * cp + 2, n0 : n0 + 256],
                                    start=first,
                                    stop=(wl_ == 1 and cp == 3),
                                    perf_mode=mybir.MatmulPerfMode.DoubleRow,
                                )
                                first = False
                    # stage to SBUF bf16 (runs inside the xl DMA window)
                    d16 = q16[hp] if kind == 0 else k16[hp]
                    nc.vector.tensor_copy(d16[:, 512 * tq : 512 * tq + 512], pt[:])

                def emit_qk_lo(hp, kind, tq, fp8_eng):
                    # xl*wh chain, then add into the staged bf16 (one PSUM
                    # operand), then fp8 copy
                    w_sb = wq_sb if kind == 0 else wk_sb
                    d8 = q8[hp] if kind == 0 else k8[hp]
                    d16 = q16[hp] if kind == 0 else k16[hp]
                    pl = mmps.tile([128, 512], F32, name="qkps", tag="qk")
                    for half in range(2):
                        n0 = 512 * tq + 256 * half
                        wr = w_sb[hp][:, 0:1024].rearrange("p (c m) -> p c m", c=8)
                        for cp in range(4):
                            nc.tensor.matmul(
                                pl[:, 256 * half : 256 * half + 256],
                                wr[:, 2 * cp : 2 * cp + 2, :],
                                xl_r[:, 2 * cp : 2 * cp + 2, n0 : n0 + 256],
                                start=(cp == 0),
                                stop=(cp == 3),
                                perf_mode=mybir.MatmulPerfMode.DoubleRow,
                            )
                    n0 = 512 * tq
                    nc.vector.tensor_add(
                        d16[:, n0 : n0 + 512], d16[:, n0 : n0 + 512], pl[:]
                    )
                    fp8_eng(d8[:, n0 : n0 + 512], d16[:, n0 : n0 + 512])

                def emit_qk_group(hp, kind, tq):
                    """q or k for head pair hp, 512 t-cols starting 512*tq.
                    Compensated fp8: xh*wh + xh*wl + xl*wh, DR chunk pairs.
                    One DVE fp8 copy + one Pool bf16 copy per group."""
                    w_sb = wq_sb if kind == 0 else wk_sb
                    d8 = q8[hp] if kind == 0 else k8[hp]
                    d16 = q16[hp] if kind == 0 else k16[hp]
                    pt = mmps.tile([128, 512], F32, name="qkps", tag="qk")
                    for half in range(2):
                        n0 = 512 * tq + 256 * half
                        first = True
                        for xs, wl_ in ((xh_r, 0), (xh_r, 1), (xl_r, 0)):
                            wr = w_sb[hp][:, 1024 * wl_ : 1024 * wl_ + 1024].rearrange(
                                "p (c m) -> p c m", c=8)
                            for cp in range(4):
                                nc.tensor.matmul(
                                    pt[:, 256 * half : 256 * half + 256],
                                    wr[:, 2 * cp : 2 * cp + 2, :],
                                    xs[:, 2 * cp : 2 * cp + 2, n0 : n0 + 256],
                                    start=first,
                                    stop=(wl_ == 0 and xs is xl_r and cp == 3),
                                    perf_mode=mybir.MatmulPerfMode.DoubleRow,
                                )
                                first = False
                    n0 = 512 * tq
                    # Pool cannot read PSUM: bf16 from PSUM on DVE, then
                    # fp8 from the bf16 copy on Pool (SBUF->SBUF)
                    nc.vector.tensor_copy(d16[:, n0 : n0 + 512], pt[:])
                    nc.gpsimd.tensor_copy(
                        d8[:, n0 : n0 + 512], d16[:, n0 : n0 + 512]
                    )

                def emit_v_t(t):
                    pv = mmps.tile([128, 512], F32, name="vps", tag="qk")[:, 0:256]
                    first = True
                    for xs, wl_ in ((xh_r, 0), (xh_r, 1), (xl_r, 0)):
                        wr = wv_sb[:, 2048 * wl_ : 2048 * wl_ + 2048].rearrange(
                            "p (c m) -> p c m", c=8)
                        for cp in range(4):
                            nc.tensor.matmul(
                                pv[:],
                                xs[:, 2 * cp : 2 * cp + 2, 128 * t : 128 * t + 128],
                                wr[:, 2 * cp : 2 * cp + 2, :],
                                start=first,
                                stop=(wl_ == 0 and xs is xl_r and cp == 3),
                                perf_mode=mybir.MatmulPerfMode.DoubleRow,
                            )
                            first = False
                    # v = pv / WS, bf16
                    nc.vector.tensor_scalar_mul(
                        v_sb[t].rearrange("p (h c) -> p h c", h=4)[:, :, 0:64],
                        pv[:].rearrange("p (h c) -> p h c", h=4),
                        1.0 / WS,
                    )

                # PE warm-up on the mask tile while input DMAs land.
                warm = sps.tile([128, 1024], F32, name="warm", tag="s")
                for i in range(13):
                    nc.tensor.matmul(
                        warm[:, 0:128], mask_sb[:], mask_sb[:],
                        start=True, stop=True,
                    )
                # Phase A: hi chains + SBUF staging of q groups 0,1 and k
                # group 0 run inside the xl DMA window; lo chains + combines
                # follow once xl lands.
                emit_qk_hi(0, 0, 0)
                emit_qk_hi(0, 0, 1)
                emit_qk_hi(0, 1, 0)
                emit_qk_lo(0, 0, 0, nc.gpsimd.tensor_copy)
                emit_qk_lo(0, 0, 1, nc.gpsimd.tensor_copy)
                emit_qk_lo(0, 1, 0, nc.vector.tensor_copy)
                for hp in range(2):
                    nc.gpsimd.dma_start(wpT_sb[hp][:], wpT_d[hp])
                # Phase B head: interleave remaining q groups with the
                # early score parts of tiles 0..3
                emit_scores_tk(0, 0, sps, only_part=0)
                emit_qk_group(0, 0, 2)
                emit_scores_tk(0, 1, sps, only_part=0)
                emit_qk_group(0, 0, 3)
                emit_scores_tk(0, 2, sps, only_part=0)
                emit_scores_tk(0, 0, sps, only_part=1)
                emit_scores_tk(0, 3, sps, only_part=0)
                emit_scores_tk(0, 1, sps, only_part=1)
                emit_qk_group(1, 0, 0)
                emit_scores_tk(0, 2, sps, only_part=1)
                emit_scores_tk(0, 3, sps, only_part=1)
                for g in range(1, 4):
                    emit_qk_group(0, 1, g)
                    emit_scores_tk(0, 4 * g, sps)
                    emit_scores_tk(0, 4 * g + 1, sps)
                    emit_qk_group(1, 0, g)
                    emit_scores_tk(0, 4 * g + 2, sps)
                    emit_scores_tk(0, 4 * g + 3, sps)
                # Phase C: k(hp1) + scores h1 + v
                for g in range(4):
                    emit_qk_group(1, 1, g)
                    emit_scores_tk(1, 4 * g, sps)
                    emit_v_t(4 * g)
                    emit_scores_tk(1, 4 * g + 1, sps)
                    emit_v_t(4 * g + 1)
                    emit_scores_tk(1, 4 * g + 2, sps)
                    emit_v_t(4 * g + 2)
                    emit_scores_tk(1, 4 * g + 3, sps)
                    emit_v_t(4 * g + 3)
                # Phase D: scores h2 + AV pair hp0 + transposes hp0,
                # score parts split around AV so ACT is fed evenly
                for g in range(4):
                    for i in range(4):
                        tk = 4 * g + i
                        emit_scores_tk(2, tk, sps, only_part=0)
                        emit_av_pair(0, tk)
                        emit_scores_tk(2, tk, sps, only_part=1)
                    emit_transpose_group(0, g, mmps, "qk")

            # ---- tail scope: scores h3 + AV hp1 + proj ----
            with (
                tc.tile_pool(name="sps2", bufs=2, space="PSUM") as sps2,
                tc.tile_pool(name="pps", bufs=2, space="PSUM") as pps,
                tc.tile_pool(name="outs", bufs=8) as outs,
            ):
                def emit_proj_t(t):
                    pps_t = {}
                    for n in range(2):
                        pp = pps.tile([128, 512], F32, name="pp", tag="p")
                        pps_t[n] = pp
                        nc.tensor.matmul(
                            pp[:],
                            yT_cat[:, 128 * t : 128 * t + 128],
                            wpT_sb[0][:, 512 * n : 512 * n + 512],
                            start=True,
                            stop=False,
                            skip_group_check=True,
                        )
                    for n in range(2):
                        pp = pps_t[n]
                        nc.tensor.matmul(
                            pp[:],
                            yT_cat[:, T + 128 * t : T + 128 * t + 128],
                            wpT_sb[1][:, 512 * n : 512 * n + 512],
                            start=False,
                            stop=True,
                            skip_group_check=True,
                        )
                        ot = outs.tile([128, 512], BF16, name="ot")
                        if n == 1 and t >= 8:
                            nc.scalar.copy(ot[:], pp[:])
                        else:
                            nc.vector.tensor_copy(ot[:], pp[:])
                        eng = nc.gpsimd if (t >= 14 and n == 1) else nc.sync
                        eng.dma_start(
                            out_d[
                                128 * t : 128 * t + 128,
                                512 * n : 512 * n + 512,
                            ],
                            ot[:],
                        )

                # Phase E: scores h3 (512-col psum chunks) + AV hp1 +
                # transposes hp1 + early proj pairs
                for g in range(4):
                    for i in range(4):
                        j = 4 * g + i
                        emit_scores_tk(3, j, sps2, part_w=1024, only_part=0)
                        emit_av_pair(1, j)
                        emit_scores_tk(3, j, sps2, part_w=1024, only_part=1)
                        # per-tile transpose: shortens the post-exp tail
                        tp = yps.tile([128, 512], F32, name="ytp_t", tag="y")
                        nc.tensor.matmul(
                            tp[:, 0:128],
                            y_sb[1][:, 128 * j : 128 * j + 128],
                            ident_sb[:],
                            start=True,
                            stop=True,
                            skip_group_check=True,
                        )
                        nc.vector.tensor_copy(
                            yT_cat[:, T + 128 * j : T + 128 * j + 128],
                            tp[:, 0:128],
                        )
                        if g >= 1:
                            emit_proj_t(4 * (g - 1) + i)
                # Phase F: remaining proj
                for t in range(12, 16):
                    emit_proj_t(t)

    nc.finalize()
    return nc


def _get_nc():
    global _cached_nc
    if _cached_nc is None:
        _cached_nc = _build()
    return _cached_nc


def kernel(x, Wq, Wk, Wv, Wp, bp):
    global last_results
    x = np.asarray(x, dtype=np.float32)
    Wq = np.asarray(Wq, dtype=np.float32)
    Wk = np.asarray(Wk, dtype=np.float32)
    Wv = np.asarray(Wv, dtype=np.float32)
    Wp = np.asarray(Wp, dtype=np.float32)
    bp = np.asarray(bp, dtype=np.float32)

    E4NP = ml_dtypes.float8_e4m3
    WpT = np.ascontiguousarray(Wp.T)  # [C_in(features), C_out]
    mask01 = np.triu(np.ones((128, 128), dtype=np.float32)).astype(ml_dtypes.bfloat16)
    ident = np.eye(128, dtype=np.float32).astype(ml_dtypes.bfloat16)

    def chunked(w):
        # [C, m] -> [128, 8*m]: c-chunk c at cols [m*c : m*(c+1)]
        m = w.shape[1]
        return np.ascontiguousarray(
            w.reshape(8, 128, m).transpose(1, 0, 2).reshape(128, 8 * m)
        )

    def hilo(a):
        hi = a.astype(E4NP)
        lo = (a - hi.astype(np.float32)).astype(E4NP)
        return hi, lo

    xT_by_batch = [np.ascontiguousarray(x[b].T) for b in range(B)]
    xhl_by_batch = [hilo(chunked(xT_by_batch[b])) for b in range(B)]

    in_maps = []
    for core in range(N_CORES):
        b, g = core // 4, core % 4
        h0 = HPC * g
        def wq_pair(W):
            res = []
            for hp in range(2):
                cat = np.concatenate(
                    [W[h0 + 2 * hp], W[h0 + 2 * hp + 1]], axis=1
                ) * WS
                h_, l_ = hilo(chunked(cat))
                res.append(np.concatenate([h_, l_], axis=1))
            return np.stack(res)
        wq_p = wq_pair(Wq)
        wk_p = wq_pair(Wk)
        wv_cat = np.concatenate([Wv[h0 + j] for j in range(HPC)], axis=1) * WS
        wvh, wvl = hilo(chunked(wv_cat))
        wv_p = np.concatenate([wvh, wvl], axis=1)
        wpT_p = np.ascontiguousarray(
            WpT[256 * g : 256 * (g + 1)].reshape(2, 128, C)
        ).astype(ml_dtypes.bfloat16)
        xh, xl = xhl_by_batch[b]
        in_maps.append(
            {
                "xh": xh, "xl": xl,
                "wq": wq_p, "wk": wk_p, "wv": wv_p,
                "wpT": wpT_p,
                "mask": mask01,
                "ident": ident,
            }
        )

    nc = _get_nc()
    kwargs = {}
    if os.environ.get("KERNEL_TRACE", "0") == "1":
        kwargs = dict(trace=True, trace_cores=list(range(N_CORES)),
                      stitch_traces=True)
    try:
        res = run_bass_kernel_spmd(
            nc, in_maps, core_ids=list(range(N_CORES)), **kwargs
        )
    except ModuleNotFoundError:
        res = run_bass_kernel_spmd(nc, in_maps, core_ids=list(range(N_CORES)))
    last_results = res

    out = np.zeros((B, T, C), dtype=np.float32)
    for core in range(N_CORES):
        b = core // 4
        out[b] += res.results[core]["out"].astype(np.float32)
    out += bp[None, None, :]
    return out
